# revision 34
# baseline (speedup 1.0000x reference)
"""Trainium2 Bass kernel for nn_Attention_11055245820093.

Swin-style attention block: qkv proj -> per-head scaled dot-product attention
with 2D relative position bias (CLS zero-padded), per-head softplus temperature,
patch-diagonal mask -> proj.

Strategy: data-parallel over batch B=64 across 8 NeuronCores (8 batches/core).
All compute per core runs in a "transposed" layout (channels on partitions,
tokens on the free dim) so no on-device transposes are needed.

Numerics (measured rel_err 1.55e-2 vs the 2e-2 gate; inputs are
deterministic):
  - QK/V projections in fp8e4m3 DoubleRow: W_hi*x_hi pair terms plus
    interleaved cross terms (W_lo*x_hi + W_hi*x_lo per k-tile).  The QK
    proj drops the cross terms of k-tiles {0, 3} -- each dropped k-tile
    trades ~7e-3 of (quadrature) error for 3.9us of PE time.
  - Attention (QK^T, exp, bias, AV, softmax divide) in bf16: any
    UNcompensated e4m3 activation quantization alone costs ~2.5e-2
    max-rel, so fp8 attention is not affordable.
  - Output proj in fp8 DoubleRow with BOTH sides compensated: wpj hi/lo
    pairs (host) x attn hi/lo pairs (split on GPSIMD after the softmax
    divide; attn is scaled by 64 via the V-path so fp8 is out of the
    subnormal range).  9 DR instructions = 4/3x the bf16 rate.
  - Output staged/DMA'd in bf16 (adds ~2e-3), final f32 convert on host.

Performance structure (112.8us total, PE busy 100us at 88.6%):
  - ONE flat software-pipelined loop over (batch, head-pair): iteration g
    issues S[g] (QK^T), then PE "filler" units, then AV[g-1] -- AV runs one
    iteration behind so its exp/bias-mul chain (ACT+Pool) has a full
    iteration of slack.
  - Batch-major order completes attention token-windows early, so
    output-proj units become mid-loop PE filler instead of a tail burst.
  - PE filler units follow a static earliest-deadline schedule: V-proj
    units B[b] (due before AV[b]), QK-proj window subtiles (due before the
    first S that reads them; K windows due one batch-pair early), and
    output-proj groups D(nt, mt) (scheduled as LATE as allowed -- the
    endgame iterations have no other filler).  The last window's D groups
    emit their first 6 DR terms before the final AV so the drain stays
    PE-busy.
  - x is stored once, flat (channel-pair-major, token columns padded to
    1584 for the 16B DoubleRow stride rule); V-proj takes arbitrary
    token-window slices of it as the stationary operand, QK-proj takes
    394-token moving slices (9 matmuls per subtile).
  - bias multiply (e = exp(S) * ebias) and the attn hi/lo fp8 split run
    on GPSIMD/Pool (proxy library tensor_tensor, SBUF-only); softmax
    reciprocal+broadcast run per-head so DVE and Pool pipeline; evictions
    alternate ACT/DVE 5:3.
  - PSUM: 2-bank unit pool (double-buffered) + 3x2-bank attention pool
    (S-jt0, S-jt1, AV+denominator rotate); prologue V-proj units borrow
    the idle attention banks to run 6-deep.
  - big DMAs split fine-grained across the SP/ACT/Pool queues in
    consumption order (1.7us init latency + 500ns min per transfer).
"""

import os
import sys

sys.path.insert(0, "/opt/trn_rl_repo")
os.environ.setdefault("MYCRO_LOCAL_CACHE", "1")

import numpy as np
import ml_dtypes

BF16 = ml_dtypes.bfloat16
F8 = ml_dtypes.float8_e4m3fn

# Problem constants (hardcoded per contract)
B, N, C, H, D = 64, 197, 768, 12, 64
NCORES = 8
BPC = B // NCORES          # 8 batches per core
T = BPC * N                # 1576 tokens per core
KT = C // 128              # 6 contraction tiles of 128
NT = 4                     # token n-tiles
TN = T // NT               # 394 tokens per n-tile
SCALE = D ** -0.5
JROWS = (128, N - 128)     # 128, 69
N2 = 2 * N
SVF = 64.0                 # wv host-scale; ones column matches so the
                           # softmax divide cancels it exactly
SQ, SK, SP = 256.0, 64.0, 64.0
TQ = 1584                  # flat x / attn pad (pair step 16-aligned)

_CACHE = {}

TRACE = False
LAST_RESULTS = None


def _build(finalize=True):
    import concourse.bass as bass
    import concourse.tile as tile
    from concourse import bacc, library_config, mybir

    dt = mybir.dt
    f32, bf16, f8 = dt.float32, dt.bfloat16, dt.float8e4
    AF = mybir.ActivationFunctionType
    OP = mybir.AluOpType
    DR = mybir.MatmulPerfMode.DoubleRow

    nc = bacc.Bacc("TRN2", target_bir_lowering=False, debug=False)

    x_hi = nc.dram_tensor(
        "x_hi", [128, KT // 2, 2, TQ], f8, kind="ExternalInput"
    ).ap()
    x_x = nc.dram_tensor(
        "x_x", [128, KT, 2, TQ], f8, kind="ExternalInput"
    ).ap()
    wv_x = nc.dram_tensor(
        "wv_x", [2, 128, KT, 2, C // 2], f8, kind="ExternalInput"
    ).ap()
    wqk_hi = nc.dram_tensor(
        "wqk_hi", [128, 2 * KT, KT // 2, 2, 128], f8, kind="ExternalInput"
    ).ap()
    wqk_x = nc.dram_tensor(
        "wqk_x", [128, 2 * KT, KT, 2, 128], f8, kind="ExternalInput"
    ).ap()
    wpj8 = nc.dram_tensor(
        "wpj8", [2, 128, KT, KT // 2, 2, 128], f8, kind="ExternalInput"
    ).ap()
    bT = nc.dram_tensor("bT", [KT, 128, 4, N], bf16, kind="ExternalInput").ap()
    bqk = nc.dram_tensor("bqk", [128, 2 * KT], f32, kind="ExternalInput").ap()
    outT = nc.dram_tensor("outT", [KT, 128, T], bf16, kind="ExternalOutput").ap()


    with tile.TileContext(nc) as tc:
        from contextlib import ExitStack

        with ExitStack() as ctx:
            nc.gpsimd.load_library(library_config.proxy)
            cp = ctx.enter_context(tc.tile_pool(name="consts", bufs=1))
            psA = ctx.enter_context(tc.tile_pool(name="psA", bufs=2, space="PSUM"))
            psC = ctx.enter_context(tc.tile_pool(name="psC", bufs=3, space="PSUM"))
            wp = ctx.enter_context(tc.tile_pool(name="work", bufs=2))

            # ---- persistent SBUF tiles; DMAs in consumption order ----
            xh_sb = cp.tile([128, KT // 2, 2, TQ], f8, name="xh", tag="xh")
            xx_sb = cp.tile([128, KT, 2, TQ], f8, name="xx", tag="xx")
            wvx_sb = cp.tile([128, 2, KT, 2, C // 2], f8, name="wvx", tag="wvx")
            wqkh_sb = cp.tile(
                [128, 2 * KT, KT // 2, 2, 128], f8, name="wqkh", tag="wqkh"
            )
            wqkx_sb = cp.tile(
                [128, 2 * KT, KT, 2, 128], f8, name="wqkx", tag="wqkx"
            )
            bqk_sb = cp.tile([128, 2 * KT], f32, name="bqk", tag="bqk")
            # startup-critical DMAs, spread so each consumer unblocks at
            # its need time: B prologue needs xh0/xx0/wvx; prologue A units
            # need only the mt=0 and mt=6 slices of wqk; later windows and
            # head-pairs stream in during the loop
            nc.gpsimd.dma_start(out=wvx_sb[:, 0], in_=wv_x[0])
            nc.gpsimd.dma_start(out=wvx_sb[:, 1], in_=wv_x[1])
            # batch-0 token span lands first, then batch 1, then the rest
            nc.scalar.dma_start(out=xh_sb[:, :, :, 0:N], in_=x_hi[:, :, :, 0:N])
            nc.sync.dma_start(out=xx_sb[:, :, :, 0:N], in_=x_x[:, :, :, 0:N])
            nc.scalar.dma_start(out=xh_sb[:, :, :, N:TN], in_=x_hi[:, :, :, N:TN])
            nc.sync.dma_start(out=xx_sb[:, :, :, N:TN], in_=x_x[:, :, :, N:TN])
            # wqk mt-slices in hp-need order (both Q and K of pair hp are
            # needed by iteration hp), alternating gpsimd/sync
            for hp in range(KT):
                qa = nc.gpsimd if hp % 2 == 0 else nc.sync
                for m in (hp, KT + hp):
                    qa.dma_start(out=wqkh_sb[:, m], in_=wqk_hi[:, m])
                    qa.dma_start(out=wqkx_sb[:, m], in_=wqk_x[:, m])
            nc.scalar.dma_start(out=bqk_sb[:], in_=bqk[:])
            for nt in range(1, NT):
                o = nt * TN
                e_ = TQ if nt == NT - 1 else (nt + 1) * TN
                nc.sync.dma_start(out=xh_sb[:, :, :, o:e_], in_=x_hi[:, :, :, o:e_])
                nc.sync.dma_start(out=xx_sb[:, :, :, o:e_], in_=x_x[:, :, :, o:e_])
            bias_sb = [
                cp.tile([128, 4, N], bf16, name=f"bias{hp}", tag=f"bias{hp}")
                for hp in range(KT)
            ]
            wpj8_sb = cp.tile(
                [128, 2, KT, KT // 2, 2, 128], f8, name="wpj8", tag="wpj8"
            )
            TP = T + 8  # attn pair-step must be 16-byte aligned
            attnp = [
                cp.tile([128, KT // 2, 2, TP], f8, name=f"atp{kind}", tag=f"atp{kind}")
                for kind in range(2)
            ]

            # qk tiles: Q (mt 0..5), K (mt 6..11)
            qk_sb = [
                cp.tile([128, T], bf16, name=f"qk{m}", tag=f"qk{m}")
                for m in range(2 * KT)
            ]
            # V per (batch, jt): (rows, 12 heads, 65) -- 64 V cols + ones col
            v_sb = {}
            for b in range(BPC):
                for jt, rows in enumerate(JROWS):
                    t_ = cp.tile(
                        [rows, H, D + 1], bf16, name=f"v{b}_{jt}", tag=f"v{b}_{jt}"
                    )
                    nc.vector.memset(t_[:, :, D : D + 1], 1.0)
                    v_sb[(b, jt)] = t_

            evict_flip = [0]

            def evict_engine():
                evict_flip[0] += 1
                return nc.vector if evict_flip[0] % 8 in (0, 3, 6) else nc.scalar

            # ---- filler unit emitters (pure PE work + one eviction) ----
            def unit_b(b, jt, n2, psv=None):
                """V-proj quarter: one psum group -> v_sb[(b, jt)] slice."""
                rows = JROWS[jt]
                o = b * N + jt * 128
                if psv is None:
                    psv = psA.tile([128, 512], f32, tag="psA")
                for p in range(KT // 2):
                    nc.tensor.matmul(
                        psv[0:rows, 0 : C // 2],
                        xh_sb[:, p, :, o : o + rows],
                        wvx_sb[:, n2, 2 * p : 2 * p + 2, 1, :],
                        start=(p == 0),
                        stop=False,
                        perf_mode=DR,
                    )
                for k in range(KT):
                    nc.tensor.matmul(
                        psv[0:rows, 0 : C // 2],
                        xx_sb[:, k, :, o : o + rows],
                        wvx_sb[:, n2, k, :, :],
                        start=False,
                        stop=(k == KT - 1),
                        perf_mode=DR,
                    )
                eng = evict_engine()
                dst = v_sb[(b, jt)][0:rows, n2 * KT : (n2 + 1) * KT, 0:D]
                src = psv[0:rows, 0 : C // 2].rearrange("p (h d) -> p h d", h=KT)
                if eng is nc.vector:
                    nc.vector.tensor_copy(dst, src)
                else:
                    nc.scalar.activation(dst, src, AF.Copy)

            # cross terms for these k-tiles are dropped in the QK proj
            # (error-compensation budget spent for PE time; validated at
            # ~1.5e-2 total rel err vs the 2e-2 gate)
            ADROP = (0, 3)
            AKEEP = tuple(k for k in range(KT) if k not in ADROP)

            def unit_a(mt, nt, eng=None):
                """QK-proj subtile: one token window of Q or K tile mt."""
                inv_s = (1.0 / SQ) if mt < KT else (1.0 / SK)
                o = nt * TN
                ps = psA.tile([128, 512], f32, tag="psA")
                for p in range(KT // 2):
                    nc.tensor.matmul(
                        ps[:, 0:TN],
                        wqkh_sb[:, mt, p, :, :],
                        xh_sb[:, p, :, o : o + TN],
                        start=(p == 0),
                        stop=False,
                        perf_mode=DR,
                    )
                for k in AKEEP:
                    nc.tensor.matmul(
                        ps[:, 0:TN],
                        wqkx_sb[:, mt, k, :, :],
                        xx_sb[:, k, :, o : o + TN],
                        start=False,
                        stop=(k == AKEEP[-1]),
                        perf_mode=DR,
                    )
                dst = qk_sb[mt][:, nt * TN : (nt + 1) * TN]
                srcp = ps[:, 0:TN]
                if (eng or evict_engine()) is nc.vector:
                    nc.vector.tensor_scalar(
                        dst, srcp, inv_s, bqk_sb[:, mt : mt + 1],
                        OP.mult, OP.add,
                    )
                else:
                    nc.scalar.activation(
                        dst, srcp, AF.Identity,
                        bias=bqk_sb[:, mt : mt + 1], scale=inv_s,
                    )

            dq = [0]

            DTERMS = [(0, 0), (1, 0), (0, 1)]  # (weight kind, attn kind)

            def unit_d_mm(ps, nt, mt, ps_):
                """ps_ = list of pair indices p to accumulate (0..2)."""
                for ti, (wk, ak) in enumerate(DTERMS):
                    for p in ps_:
                        nc.tensor.matmul(
                            ps[:, 0:TN],
                            wpj8_sb[:, wk, mt, p, :, :],
                            attnp[ak][:, p, :, nt * TN : (nt + 1) * TN],
                            start=(ti == 0 and p == 0),
                            stop=(ti == 2 and p == 2),
                            perf_mode=DR,
                        )

            def unit_d_fin(ps, nt, mt, act=None):
                ot = wp.tile([128, TN], bf16, tag="ot", bufs=3)
                use_dve = (act is False) if act is not None else (
                    evict_engine() is nc.vector)
                if use_dve:
                    nc.vector.tensor_scalar_mul(ot[:], ps[:, 0:TN], 1.0 / 4096.0)
                else:
                    nc.scalar.activation(ot[:], ps[:, 0:TN], AF.Copy,
                                         scale=1.0 / 4096.0)
                dq[0] += 1
                nc.sync.dma_start(
                    out=outT[mt, :, nt * TN : (nt + 1) * TN], in_=ot[:]
                )

            def unit_d(nt, mt):
                """output-proj group: one (window, out-tile) -> outT DMA."""
                ps = psA.tile([128, 512], f32, tag="psA")
                unit_d_mm(ps, nt, mt, (0, 1, 2))
                unit_d_fin(ps, nt, mt)

            # ---- attention pieces ----
            s_tiles = {}

            def emit_s(g, e2):
                """S^T matmuls + exp + Pool bias-mul, per key-window."""
                b, hp = divmod(g, KT)
                for jt, rows in enumerate(JROWS):
                    ps = psC.tile([128, 2, 512], f32, tag="psC")
                    for hh in range(2):
                        base = 64 * hh
                        i0 = b * N + jt * 128
                        nc.tensor.matmul(
                            ps[0:rows, hh, 0:N],
                            qk_sb[KT + hp][base : base + 64, i0 : i0 + rows],
                            qk_sb[hp][base : base + 64, b * N : (b + 1) * N],
                            start=True,
                            stop=True,
                        )
                    eu = wp.tile([128, 2, N], bf16, tag=f"eu{jt}", bufs=3)
                    nc.scalar.activation(
                        eu[0:rows, :, :], ps[0:rows, :, 0:N], AF.Exp
                    )
                    nc.gpsimd.tensor_mul(
                        e2[0:rows, :, jt, :],
                        eu[0:rows, :, :],
                        bias_sb[hp][0:rows, 2 * jt : 2 * jt + 2, :],
                    )

            def emit_av(g):
                """AV + softmax normalize for iteration g (runs at g+1)."""
                b, hp = divmod(g, KT)
                e2 = s_tiles.pop(g)
                po = psC.tile([128, 2, 512], f32, tag="psC")
                for hh in range(2):
                    h = 2 * hp + hh
                    for jt, rows in enumerate(JROWS):
                        nc.tensor.matmul(
                            po[0 : D + 1, hh, 0:N],
                            v_sb[(b, jt)][0:rows, h, 0 : D + 1],
                            e2[0:rows, hh, jt, :],
                            start=(jt == 0),
                            stop=(jt == 1),
                        )
                # per-head recip/bcast/mul chains pipeline DVE and Pool,
                # shortening the latency until po's PSUM banks free up
                r2 = wp.tile([1, 2, N], bf16, tag="r2", bufs=3)
                rb = wp.tile([128, N2], bf16, tag="rb", bufs=3)
                at = wp.tile([128, N], bf16, tag="atmp", bufs=3)
                with nc.allow_low_precision(
                    reason="softmax denom reciprocal in bf16"
                ):
                    nc.vector.reciprocal(r2[:, 0, :], po[D : D + 1, 0, 0:N])
                    nc.gpsimd.partition_broadcast(rb[:, 0:N], r2[:, 0, :])
                    nc.vector.reciprocal(r2[:, 1, :], po[D : D + 1, 1, 0:N])
                    nc.gpsimd.partition_broadcast(rb[:, N:N2], r2[:, 1, :])
                nc.vector.tensor_mul(at[0:D, :], po[0:D, 0, 0:N], rb[0:D, 0:N])
                nc.vector.tensor_mul(
                    at[D : 2 * D, :], po[0:D, 1, 0:N], rb[D : 2 * D, N:N2]
                )
                # fp8 hi/lo split of attn^T (feeds the DR output proj)
                nc.gpsimd.tensor_copy(
                    attnp[0][:, hp // 2, hp % 2, b * N : (b + 1) * N], at[:]
                )
                nc.gpsimd.tensor_sub(
                    attnp[1][:, hp // 2, hp % 2, b * N : (b + 1) * N],
                    at[:],
                    attnp[0][:, hp // 2, hp % 2, b * N : (b + 1) * N],
                )

            # ---- static filler schedule ----
            # AB units have HARD deadlines (due = first iteration whose S or
            # AV reads their output; emitting later would cycle the in-order
            # PE queue through an ACT/DVE eviction that sits behind stalled
            # work -> deadlock).  D units instead have a READY iteration
            # (earliest emission keeping attn writes ahead in PE order).
            abunits = []
            for b in range(2, BPC):
                for jt in range(2):
                    for n2 in range(2):
                        # V[b] consumed by AV[b, hp0] emitted at iter 6b+1
                        abunits.append(
                            (max(0, KT * b - 1),
                             lambda b=b, jt=jt, n2=n2: unit_b(b, jt, n2))
                        )
            for hp in range(KT):
                for nt in range(NT):
                    for mt in (hp, KT + hp):
                        if hp == 0 and nt == 0:
                            continue  # prologue
                        # first S reading window nt of pair hp: b = 2nt
                        abunits.append(
                            (2 * nt * KT + hp,
                             lambda mt=mt, nt=nt: unit_a(mt, nt))
                        )
            abunits.sort(key=lambda u: u[0])
            # D(nt) is ready from iteration 12nt+13 (window nt's last norms
            # are emitted during iteration 12nt+12).  Schedule the units as
            # LATE as allowed: the endgame iterations (>=42) have no A/B
            # units left (all deadlines passed) and otherwise starve PE.
            DSLOT = {0: (15, 17, 19, 21, 23, 25),
                     1: (27, 30, 33, 36, 39, 42),
                     2: (43, 44, 45, 46, 47, 47)}
            DSLOT = {0: (15, 17, 19, 21, 23, 25),
                     1: (27, 30, 34, 38, 42, 44),
                     2: (44, 45, 46, 46, 47, 47)}
            dsched = {}
            for nt in range(NT - 1):
                for mt in range(KT):
                    dsched.setdefault(DSLOT[nt][mt], []).append(
                        lambda nt=nt, mt=mt: unit_d(nt, mt)
                    )

            # ---- prologue ----
            for hp in range(KT):
                (nc.scalar if hp % 2 == 0 else nc.gpsimd).dma_start(
                    out=bias_sb[hp][:], in_=bT[hp]
                )
            nc.sync.dma_start(out=wpj8_sb[:, 0], in_=wpj8[0])
            nc.sync.dma_start(out=wpj8_sb[:, 1], in_=wpj8[1])
            # prologue B units run 2-per-psC-tile (6 banks idle here), so
            # PE isn't paced by the psA double-buffer eviction latency
            for b in range(2):
                for jt in range(2):
                    t_ = psC.tile([128, 2, 512], f32, tag="psC")
                    for n2 in range(2):
                        unit_b(b, jt, n2, psv=t_[:, n2])
                if b == 0:
                    unit_a(0, 0)
                    unit_a(KT, 0)

            # ---- flat pipelined loop ----
            NITER = KT * BPC
            TARGET = 1250  # ns of PE filler per iteration
            ABCOST = 700   # avg A/B unit PE ns
            ai = 0
            for g in range(NITER + 1):
                # hard-due AB units must precede S[g]
                spent = 0
                while ai < len(abunits) and abunits[ai][0] <= g:
                    abunits[ai][1]()
                    ai += 1
                    spent += ABCOST
                if g < NITER:
                    e2 = wp.tile([128, 2, 2, N], bf16, tag="e2", bufs=3)
                    s_tiles[g] = e2
                    emit_s(g, e2)
                # scheduled D units + EDF top-up between S[g] and AV[g-1]
                for fn in dsched.get(g, ()):
                    fn()
                    spent += 738
                while spent < TARGET and ai < len(abunits):
                    abunits[ai][1]()
                    ai += 1
                    spent += ABCOST
                if g == NITER:
                    # last window: pair p=0,1 partials only need head-pairs
                    # 0..3; they fill PE while the final AV/normalize chain
                    # completes.  Two extra 2-bank psC tiles host one output
                    # tile's group per bank.
                    d3 = []
                    for mt in range(2):
                        t_ = psA.tile([128, 512], f32, tag="psA")
                        d3.append(t_[:])
                        unit_d_mm(t_[:], NT - 1, mt, (0, 1))
                if g >= 1:
                    emit_av(g - 1)
            for mt in range(2):
                unit_d_mm(d3[mt], NT - 1, mt, (2,))
                unit_d_fin(d3[mt], NT - 1, mt, act=True)
            for mt in range(2, KT):
                ps = psA.tile([128, 512], f32, tag="psA")
                unit_d_mm(ps, NT - 1, mt, (0, 1, 2))
                unit_d_fin(ps, NT - 1, mt, act=True)

    if finalize:
        nc.finalize()
    return nc


def _split8(a):
    """Error-compensated fp8 pair: a ~= hi + lo, each e4m3."""
    hi = a.astype(F8)
    lo = (a - hi.astype(np.float32)).astype(F8)
    return hi, lo


def _ktiles(a, nf):
    """(768, nf) -> (128, KT, nf) partition-major k-tiles."""
    return np.ascontiguousarray(a.reshape(KT, 128, nf).transpose(1, 0, 2))


def _host_prep(x, qkv_w, qkv_b, proj_w, proj_b, rel_table, log_temp, rel_index):
    """Build the per-core input maps (host-side layout prep only)."""
    x = np.asarray(x, np.float32)
    qkv_w = np.asarray(qkv_w, np.float32)
    qkv_b = np.asarray(qkv_b, np.float32)
    proj_w = np.asarray(proj_w, np.float32)
    rel_table = np.asarray(rel_table, np.float32)
    log_temp = np.asarray(log_temp, np.float32)
    rel_index = np.asarray(rel_index)

    temp = np.log1p(np.exp(log_temp.astype(np.float64))).astype(np.float32)
    alpha = (SCALE / temp).astype(np.float32)         # (H,) folded into q
    alpha_c = np.repeat(alpha, D)                     # (768,)

    # qk weights, host-scaled for fp8 range (SQ incl. alpha; SK plain),
    # split into hi/lo e4m3 pairs; hi-only and interleaved-cross layouts
    wqkT = qkv_w[0 : 2 * C].T.copy()                  # (768, 1536)
    wqkT[:, 0:C] *= alpha_c[None, :] * SQ
    wqkT[:, C : 2 * C] *= SK
    qhi, qlo = _split8(wqkT)
    qhi_t = _ktiles(qhi.astype(np.float32), 2 * C)
    qlo_t = _ktiles(qlo.astype(np.float32), 2 * C)
    wqk_hi_np = np.ascontiguousarray(
        qhi_t.reshape(128, KT // 2, 2, 2 * KT, 128).transpose(0, 3, 1, 2, 4)
    ).astype(F8)
    # cross weights LO-FIRST so the shared x cross buffer can stay HI-FIRST
    wqk_x_np = np.ascontiguousarray(
        np.stack([qlo_t, qhi_t], axis=2)
        .reshape(128, KT, 2, 2 * KT, 128)
        .transpose(0, 3, 1, 2, 4)
    ).astype(F8)

    # wv as fp8 hi/lo cross pairs, LO-FIRST (moving operand of V phase),
    # host-scaled by SVF out of the e4m3 subnormal range
    wvT = qkv_w[2 * C : 3 * C].T * SVF                # (768, 768)
    vhi, vlo = _split8(wvT)
    vhi_t = _ktiles(vhi.astype(np.float32), C)
    vlo_t = _ktiles(vlo.astype(np.float32), C)
    wvx = np.stack([vlo_t, vhi_t], axis=2)            # (128, KT, 2, C) lo-first
    wv_x_np = np.stack(
        [wvx[:, :, :, 0 : C // 2], wvx[:, :, :, C // 2 : C]], axis=0
    ).astype(F8)
    wpjT = proj_w.T * SP                              # (768, 768)
    phi, plo = _split8(wpjT)
    phi_t = _ktiles(phi.astype(np.float32), C).reshape(128, KT // 2, 2, C)
    plo_t = _ktiles(plo.astype(np.float32), C).reshape(128, KT // 2, 2, C)
    wpj8_np = np.stack(
        [
            np.ascontiguousarray(
                t.reshape(128, KT // 2, 2, KT, 128).transpose(0, 3, 1, 2, 4)
            )
            for t in (phi_t, plo_t)
        ],
        axis=0,
    ).astype(F8)

    bq = qkv_b[0:C] * alpha_c
    bk = qkv_b[C : 2 * C]
    bqk_np = np.concatenate([bq, bk]).reshape(2 * KT, 128).T.copy().astype(np.float32)

    # multiplicative bias table: exp((relpos bias)/temp), diag -> 0, CLS -> 1,
    # transposed to (j, i); paired layout (KT, j, 2N)
    rpb = rel_table[rel_index]                        # (196, 196, H)
    bias = np.zeros((H, N, N), np.float32)
    bias[:, 1:, 1:] = rpb.transpose(2, 0, 1) / temp[:, None, None]
    ebias = np.exp(bias)
    idx = np.arange(1, N)
    ebias[:, idx, idx] = 0.0
    ebT = ebias.transpose(0, 2, 1)                    # (H, j, i)
    bT_np = np.zeros((KT, 128, 4, N), np.float32)
    for jt, rows in enumerate(JROWS):
        blk = ebT[:, jt * 128 : jt * 128 + rows, :]   # (H, rows, N)
        bT_np[:, 0:rows, 2 * jt : 2 * jt + 2] = (
            blk.reshape(KT, 2, rows, N).transpose(0, 2, 1, 3)
        )
    bT_np = bT_np.astype(BF16)


    in_maps = []
    for c in range(NCORES):
        xc = x[c * BPC : (c + 1) * BPC].reshape(T, C).T  # (768, T)
        xhi, xlo = _split8(xc)
        xhi_t = _ktiles(xhi.astype(np.float32), T)      # (128, KT, T)
        xlo_t = _ktiles(xlo.astype(np.float32), T)
        x_hi_np = np.zeros((128, KT // 2, 2, TQ), np.float32)
        x_hi_np[:, :, :, 0:T] = xhi_t.reshape(128, KT // 2, 2, T)
        x_hi_np = x_hi_np.astype(F8)
        x_x_np = np.zeros((128, KT, 2, TQ), np.float32)
        x_x_np[:, :, :, 0:T] = np.stack([xhi_t, xlo_t], axis=2)
        x_x_np = x_x_np.astype(F8)
        in_maps.append(
            {
                "x_hi": x_hi_np,
                "x_x": x_x_np,
                "wv_x": wv_x_np,
                "wqk_hi": wqk_hi_np,
                "wqk_x": wqk_x_np,
                "wpj8": wpj8_np,
                "bT": bT_np,
                "bqk": bqk_np,
            }
        )
    return in_maps


def kernel(**inputs) -> np.ndarray:
    global LAST_RESULTS
    from concourse.bass_utils import run_bass_kernel_spmd

    if "nc" not in _CACHE:
        _CACHE["nc"] = _build()
    nc = _CACHE["nc"]

    in_maps = _host_prep(**inputs)
    try:
        res = run_bass_kernel_spmd(
            nc, in_maps, core_ids=list(range(NCORES)), trace=TRACE
        )
    except ModuleNotFoundError:
        res = run_bass_kernel_spmd(
            nc, in_maps, core_ids=list(range(NCORES)), trace=False
        )
    LAST_RESULTS = res

    # v-bias rides through attention unchanged (rows of attn sum to 1), so
    # its proj image folds into the constant output bias added here
    proj_b = np.asarray(inputs["proj_b"], np.float32)
    proj_w = np.asarray(inputs["proj_w"], np.float32)
    bv = np.asarray(inputs["qkv_b"], np.float32)[2 * C : 3 * C]
    b_eff = proj_b + proj_w @ bv
    outs = []
    for c in range(NCORES):
        oT = np.asarray(res.results[c]["outT"], np.float32).reshape(C, T)
        outs.append(oT.T.reshape(BPC, N, C))
    out = np.concatenate(outs, axis=0) + b_eff[None, None, :]
    return out.astype(np.float32)


# revision 35
# speedup vs baseline: 1.0020x; 1.0020x over previous
"""Trainium2 Bass kernel for nn_Attention_11055245820093.

Swin-style attention block: qkv proj -> per-head scaled dot-product attention
with 2D relative position bias (CLS zero-padded), per-head softplus temperature,
patch-diagonal mask -> proj.

Strategy: data-parallel over batch B=64 across 8 NeuronCores (8 batches/core).
All compute per core runs in a "transposed" layout (channels on partitions,
tokens on the free dim) so no on-device transposes are needed.

Numerics (measured rel_err 1.55e-2 vs the 2e-2 gate; inputs are
deterministic):
  - QK/V projections in fp8e4m3 DoubleRow: W_hi*x_hi pair terms plus
    interleaved cross terms (W_lo*x_hi + W_hi*x_lo per k-tile).  The QK
    proj drops the cross terms of k-tiles {0, 3} -- each dropped k-tile
    trades ~7e-3 of (quadrature) error for 3.9us of PE time.
  - Attention (QK^T, exp, bias, AV, softmax divide) in bf16: any
    UNcompensated e4m3 activation quantization alone costs ~2.5e-2
    max-rel, so fp8 attention is not affordable.
  - Output proj in fp8 DoubleRow with BOTH sides compensated: wpj hi/lo
    pairs (host) x attn hi/lo pairs (split on GPSIMD after the softmax
    divide; attn is scaled by 64 via the V-path so fp8 is out of the
    subnormal range).  9 DR instructions = 4/3x the bf16 rate.
  - Output staged/DMA'd in bf16 (adds ~2e-3), final f32 convert on host.

Performance structure (112.8us total, PE busy 100us at 88.6%):
  - ONE flat software-pipelined loop over (batch, head-pair): iteration g
    issues S[g] (QK^T), then PE "filler" units, then AV[g-1] -- AV runs one
    iteration behind so its exp/bias-mul chain (ACT+Pool) has a full
    iteration of slack.
  - Batch-major order completes attention token-windows early, so
    output-proj units become mid-loop PE filler instead of a tail burst.
  - PE filler units follow a static earliest-deadline schedule: V-proj
    units B[b] (due before AV[b]), QK-proj window subtiles (due before the
    first S that reads them; K windows due one batch-pair early), and
    output-proj groups D(nt, mt) (scheduled as LATE as allowed -- the
    endgame iterations have no other filler).  The last window's D groups
    emit their first 6 DR terms before the final AV so the drain stays
    PE-busy.
  - x is stored once, flat (channel-pair-major, token columns padded to
    1584 for the 16B DoubleRow stride rule); V-proj takes arbitrary
    token-window slices of it as the stationary operand, QK-proj takes
    394-token moving slices (9 matmuls per subtile).
  - bias multiply (e = exp(S) * ebias) and the attn hi/lo fp8 split run
    on GPSIMD/Pool (proxy library tensor_tensor, SBUF-only); softmax
    reciprocal+broadcast run per-head so DVE and Pool pipeline; evictions
    alternate ACT/DVE 5:3.
  - PSUM: 2-bank unit pool (double-buffered) + 3x2-bank attention pool
    (S-jt0, S-jt1, AV+denominator rotate); prologue V-proj units borrow
    the idle attention banks to run 6-deep.
  - big DMAs split fine-grained across the SP/ACT/Pool queues in
    consumption order (1.7us init latency + 500ns min per transfer).
"""

import os
import sys

sys.path.insert(0, "/opt/trn_rl_repo")
os.environ.setdefault("MYCRO_LOCAL_CACHE", "1")

import numpy as np
import ml_dtypes

BF16 = ml_dtypes.bfloat16
F8 = ml_dtypes.float8_e4m3fn

# Problem constants (hardcoded per contract)
B, N, C, H, D = 64, 197, 768, 12, 64
NCORES = 8
BPC = B // NCORES          # 8 batches per core
T = BPC * N                # 1576 tokens per core
KT = C // 128              # 6 contraction tiles of 128
NT = 4                     # token n-tiles
TN = T // NT               # 394 tokens per n-tile
SCALE = D ** -0.5
JROWS = (128, N - 128)     # 128, 69
N2 = 2 * N
SVF = 64.0                 # wv host-scale; ones column matches so the
                           # softmax divide cancels it exactly
SQ, SK, SP = 256.0, 64.0, 64.0
TQ = 1584                  # flat x / attn pad (pair step 16-aligned)

_CACHE = {}

TRACE = False
LAST_RESULTS = None


def _build(finalize=True):
    import concourse.bass as bass
    import concourse.tile as tile
    from concourse import bacc, library_config, mybir

    dt = mybir.dt
    f32, bf16, f8 = dt.float32, dt.bfloat16, dt.float8e4
    AF = mybir.ActivationFunctionType
    OP = mybir.AluOpType
    DR = mybir.MatmulPerfMode.DoubleRow

    nc = bacc.Bacc("TRN2", target_bir_lowering=False, debug=False)

    x_hi = nc.dram_tensor(
        "x_hi", [128, KT // 2, 2, TQ], f8, kind="ExternalInput"
    ).ap()
    x_x = nc.dram_tensor(
        "x_x", [128, KT, 2, TQ], f8, kind="ExternalInput"
    ).ap()
    wv_x = nc.dram_tensor(
        "wv_x", [2, 128, KT, 2, C // 2], f8, kind="ExternalInput"
    ).ap()
    wqk_hi = nc.dram_tensor(
        "wqk_hi", [128, 2 * KT, KT // 2, 2, 128], f8, kind="ExternalInput"
    ).ap()
    wqk_x = nc.dram_tensor(
        "wqk_x", [128, 2 * KT, KT, 2, 128], f8, kind="ExternalInput"
    ).ap()
    wpj8 = nc.dram_tensor(
        "wpj8", [2, 128, KT, KT // 2, 2, 128], f8, kind="ExternalInput"
    ).ap()
    bT = nc.dram_tensor("bT", [KT, 128, 4, N], bf16, kind="ExternalInput").ap()
    bqk = nc.dram_tensor("bqk", [128, 2 * KT], f32, kind="ExternalInput").ap()
    outT = nc.dram_tensor("outT", [KT, 128, T], bf16, kind="ExternalOutput").ap()


    with tile.TileContext(nc) as tc:
        from contextlib import ExitStack

        with ExitStack() as ctx:
            nc.gpsimd.load_library(library_config.proxy)
            cp = ctx.enter_context(tc.tile_pool(name="consts", bufs=1))
            psA = ctx.enter_context(tc.tile_pool(name="psA", bufs=2, space="PSUM"))
            psC = ctx.enter_context(tc.tile_pool(name="psC", bufs=3, space="PSUM"))
            wp = ctx.enter_context(tc.tile_pool(name="work", bufs=2))

            # ---- persistent SBUF tiles; DMAs in consumption order ----
            xh_sb = cp.tile([128, KT // 2, 2, TQ], f8, name="xh", tag="xh")
            xx_sb = cp.tile([128, KT, 2, TQ], f8, name="xx", tag="xx")
            wvx_sb = cp.tile([128, 2, KT, 2, C // 2], f8, name="wvx", tag="wvx")
            wqkh_sb = cp.tile(
                [128, 2 * KT, KT // 2, 2, 128], f8, name="wqkh", tag="wqkh"
            )
            wqkx_sb = cp.tile(
                [128, 2 * KT, KT, 2, 128], f8, name="wqkx", tag="wqkx"
            )
            bqk_sb = cp.tile([128, 2 * KT], f32, name="bqk", tag="bqk")
            # startup-critical DMAs, spread so each consumer unblocks at
            # its need time: B prologue needs xh0/xx0/wvx; prologue A units
            # need only the mt=0 and mt=6 slices of wqk; later windows and
            # head-pairs stream in during the loop
            nc.gpsimd.dma_start(out=wvx_sb[:, 0], in_=wv_x[0])
            nc.gpsimd.dma_start(out=wvx_sb[:, 1], in_=wv_x[1])
            # batch-0 token span lands first, then batch 1, then the rest
            nc.scalar.dma_start(out=xh_sb[:, :, :, 0:N], in_=x_hi[:, :, :, 0:N])
            nc.sync.dma_start(out=xx_sb[:, :, :, 0:N], in_=x_x[:, :, :, 0:N])
            nc.scalar.dma_start(out=xh_sb[:, :, :, N:TN], in_=x_hi[:, :, :, N:TN])
            nc.sync.dma_start(out=xx_sb[:, :, :, N:TN], in_=x_x[:, :, :, N:TN])
            # wqk mt-slices in hp-need order (both Q and K of pair hp are
            # needed by iteration hp), alternating gpsimd/sync
            for hp in range(KT):
                qa = nc.gpsimd if hp % 2 == 0 else nc.sync
                for m in (hp, KT + hp):
                    qa.dma_start(out=wqkh_sb[:, m], in_=wqk_hi[:, m])
                    qa.dma_start(out=wqkx_sb[:, m], in_=wqk_x[:, m])
            nc.scalar.dma_start(out=bqk_sb[:], in_=bqk[:])
            for nt in range(1, NT):
                o = nt * TN
                e_ = TQ if nt == NT - 1 else (nt + 1) * TN
                nc.sync.dma_start(out=xh_sb[:, :, :, o:e_], in_=x_hi[:, :, :, o:e_])
                nc.sync.dma_start(out=xx_sb[:, :, :, o:e_], in_=x_x[:, :, :, o:e_])
            bias_sb = [
                cp.tile([128, 4, N], bf16, name=f"bias{hp}", tag=f"bias{hp}")
                for hp in range(KT)
            ]
            wpj8_sb = cp.tile(
                [128, 2, KT, KT // 2, 2, 128], f8, name="wpj8", tag="wpj8"
            )
            TP = T + 8  # attn pair-step must be 16-byte aligned
            attnp = [
                cp.tile([128, KT // 2, 2, TP], f8, name=f"atp{kind}", tag=f"atp{kind}")
                for kind in range(2)
            ]

            # qk tiles: Q (mt 0..5), K (mt 6..11)
            qk_sb = [
                cp.tile([128, T], bf16, name=f"qk{m}", tag=f"qk{m}")
                for m in range(2 * KT)
            ]
            # V per (batch, jt): (rows, 12 heads, 65) -- 64 V cols + ones col
            v_sb = {}
            for b in range(BPC):
                for jt, rows in enumerate(JROWS):
                    t_ = cp.tile(
                        [rows, H, D + 1], bf16, name=f"v{b}_{jt}", tag=f"v{b}_{jt}"
                    )
                    nc.vector.memset(t_[:, :, D : D + 1], 1.0)
                    v_sb[(b, jt)] = t_

            evict_flip = [0]

            def evict_engine():
                evict_flip[0] += 1
                return nc.vector if evict_flip[0] % 2 == 0 else nc.scalar

            # ---- filler unit emitters (pure PE work + one eviction) ----
            def unit_b(b, jt, n2, psv=None):
                """V-proj quarter: one psum group -> v_sb[(b, jt)] slice."""
                rows = JROWS[jt]
                o = b * N + jt * 128
                if psv is None:
                    psv = psA.tile([128, 512], f32, tag="psA")
                for p in range(KT // 2):
                    nc.tensor.matmul(
                        psv[0:rows, 0 : C // 2],
                        xh_sb[:, p, :, o : o + rows],
                        wvx_sb[:, n2, 2 * p : 2 * p + 2, 1, :],
                        start=(p == 0),
                        stop=False,
                        perf_mode=DR,
                    )
                for k in range(KT):
                    nc.tensor.matmul(
                        psv[0:rows, 0 : C // 2],
                        xx_sb[:, k, :, o : o + rows],
                        wvx_sb[:, n2, k, :, :],
                        start=False,
                        stop=(k == KT - 1),
                        perf_mode=DR,
                    )
                eng = evict_engine()
                dst = v_sb[(b, jt)][0:rows, n2 * KT : (n2 + 1) * KT, 0:D]
                src = psv[0:rows, 0 : C // 2].rearrange("p (h d) -> p h d", h=KT)
                if eng is nc.vector:
                    nc.vector.tensor_copy(dst, src)
                else:
                    nc.scalar.activation(dst, src, AF.Copy)

            # cross terms for these k-tiles are dropped in the QK proj
            # (error-compensation budget spent for PE time; validated at
            # ~1.5e-2 total rel err vs the 2e-2 gate)
            ADROP = (0, 3)
            AKEEP = tuple(k for k in range(KT) if k not in ADROP)

            def unit_a(mt, nt, eng=None):
                """QK-proj subtile: one token window of Q or K tile mt."""
                inv_s = (1.0 / SQ) if mt < KT else (1.0 / SK)
                o = nt * TN
                ps = psA.tile([128, 512], f32, tag="psA")
                for p in range(KT // 2):
                    nc.tensor.matmul(
                        ps[:, 0:TN],
                        wqkh_sb[:, mt, p, :, :],
                        xh_sb[:, p, :, o : o + TN],
                        start=(p == 0),
                        stop=False,
                        perf_mode=DR,
                    )
                for k in AKEEP:
                    nc.tensor.matmul(
                        ps[:, 0:TN],
                        wqkx_sb[:, mt, k, :, :],
                        xx_sb[:, k, :, o : o + TN],
                        start=False,
                        stop=(k == AKEEP[-1]),
                        perf_mode=DR,
                    )
                dst = qk_sb[mt][:, nt * TN : (nt + 1) * TN]
                srcp = ps[:, 0:TN]
                if (eng or evict_engine()) is nc.vector:
                    nc.vector.tensor_scalar(
                        dst, srcp, inv_s, bqk_sb[:, mt : mt + 1],
                        OP.mult, OP.add,
                    )
                else:
                    nc.scalar.activation(
                        dst, srcp, AF.Identity,
                        bias=bqk_sb[:, mt : mt + 1], scale=inv_s,
                    )

            dq = [0]

            DTERMS = [(0, 0), (1, 0), (0, 1)]  # (weight kind, attn kind)

            def unit_d_mm(ps, nt, mt, ps_):
                """ps_ = list of pair indices p to accumulate (0..2)."""
                for ti, (wk, ak) in enumerate(DTERMS):
                    for p in ps_:
                        nc.tensor.matmul(
                            ps[:, 0:TN],
                            wpj8_sb[:, wk, mt, p, :, :],
                            attnp[ak][:, p, :, nt * TN : (nt + 1) * TN],
                            start=(ti == 0 and p == 0),
                            stop=(ti == 2 and p == 2),
                            perf_mode=DR,
                        )

            def unit_d_fin(ps, nt, mt, act=None):
                ot = wp.tile([128, TN], bf16, tag="ot", bufs=3)
                use_dve = (act is False) if act is not None else (
                    evict_engine() is nc.vector)
                if use_dve:
                    nc.vector.tensor_scalar_mul(ot[:], ps[:, 0:TN], 1.0 / 4096.0)
                else:
                    nc.scalar.activation(ot[:], ps[:, 0:TN], AF.Copy,
                                         scale=1.0 / 4096.0)
                dq[0] += 1
                nc.sync.dma_start(
                    out=outT[mt, :, nt * TN : (nt + 1) * TN], in_=ot[:]
                )

            def unit_d(nt, mt):
                """output-proj group: one (window, out-tile) -> outT DMA."""
                ps = psA.tile([128, 512], f32, tag="psA")
                unit_d_mm(ps, nt, mt, (0, 1, 2))
                unit_d_fin(ps, nt, mt)

            # ---- attention pieces ----
            s_tiles = {}

            def emit_s(g, e2):
                """S^T matmuls + exp + Pool bias-mul, per key-window."""
                b, hp = divmod(g, KT)
                for jt, rows in enumerate(JROWS):
                    ps = psC.tile([128, 2, 512], f32, tag="psC")
                    for hh in range(2):
                        base = 64 * hh
                        i0 = b * N + jt * 128
                        nc.tensor.matmul(
                            ps[0:rows, hh, 0:N],
                            qk_sb[KT + hp][base : base + 64, i0 : i0 + rows],
                            qk_sb[hp][base : base + 64, b * N : (b + 1) * N],
                            start=True,
                            stop=True,
                        )
                    eu = wp.tile([128, 2, N], bf16, tag=f"eu{jt}", bufs=3)
                    nc.scalar.activation(
                        eu[0:rows, :, :], ps[0:rows, :, 0:N], AF.Exp
                    )
                    nc.gpsimd.tensor_mul(
                        e2[0:rows, :, jt, :],
                        eu[0:rows, :, :],
                        bias_sb[hp][0:rows, 2 * jt : 2 * jt + 2, :],
                    )

            def emit_av(g):
                """AV + softmax normalize for iteration g (runs at g+1)."""
                b, hp = divmod(g, KT)
                e2 = s_tiles.pop(g)
                po = psC.tile([128, 2, 512], f32, tag="psC")
                for hh in range(2):
                    h = 2 * hp + hh
                    for jt, rows in enumerate(JROWS):
                        nc.tensor.matmul(
                            po[0 : D + 1, hh, 0:N],
                            v_sb[(b, jt)][0:rows, h, 0 : D + 1],
                            e2[0:rows, hh, jt, :],
                            start=(jt == 0),
                            stop=(jt == 1),
                        )
                # per-head recip/bcast/mul chains pipeline DVE and Pool,
                # shortening the latency until po's PSUM banks free up
                r2 = wp.tile([1, 2, N], bf16, tag="r2", bufs=3)
                rb = wp.tile([128, N2], bf16, tag="rb", bufs=3)
                at = wp.tile([128, N], bf16, tag="atmp", bufs=3)
                with nc.allow_low_precision(
                    reason="softmax denom reciprocal in bf16"
                ):
                    nc.vector.reciprocal(r2[:, 0, :], po[D : D + 1, 0, 0:N])
                    nc.gpsimd.partition_broadcast(rb[:, 0:N], r2[:, 0, :])
                    nc.vector.reciprocal(r2[:, 1, :], po[D : D + 1, 1, 0:N])
                    nc.gpsimd.partition_broadcast(rb[:, N:N2], r2[:, 1, :])
                nc.vector.tensor_mul(at[0:D, :], po[0:D, 0, 0:N], rb[0:D, 0:N])
                nc.vector.tensor_mul(
                    at[D : 2 * D, :], po[0:D, 1, 0:N], rb[D : 2 * D, N:N2]
                )
                # fp8 hi/lo split of attn^T (feeds the DR output proj)
                nc.gpsimd.tensor_copy(
                    attnp[0][:, hp // 2, hp % 2, b * N : (b + 1) * N], at[:]
                )
                nc.gpsimd.tensor_sub(
                    attnp[1][:, hp // 2, hp % 2, b * N : (b + 1) * N],
                    at[:],
                    attnp[0][:, hp // 2, hp % 2, b * N : (b + 1) * N],
                )

            # ---- static filler schedule ----
            # AB units have HARD deadlines (due = first iteration whose S or
            # AV reads their output; emitting later would cycle the in-order
            # PE queue through an ACT/DVE eviction that sits behind stalled
            # work -> deadlock).  D units instead have a READY iteration
            # (earliest emission keeping attn writes ahead in PE order).
            abunits = []
            for b in range(2, BPC):
                for jt in range(2):
                    for n2 in range(2):
                        # V[b] consumed by AV[b, hp0] emitted at iter 6b+1
                        abunits.append(
                            (max(0, KT * b - 1),
                             lambda b=b, jt=jt, n2=n2: unit_b(b, jt, n2))
                        )
            for hp in range(KT):
                for nt in range(NT):
                    for mt in (hp, KT + hp):
                        if hp == 0 and nt == 0:
                            continue  # prologue
                        # first S reading window nt of pair hp: b = 2nt
                        abunits.append(
                            (2 * nt * KT + hp,
                             lambda mt=mt, nt=nt: unit_a(mt, nt))
                        )
            abunits.sort(key=lambda u: u[0])
            # D(nt) is ready from iteration 12nt+13 (window nt's last norms
            # are emitted during iteration 12nt+12).  Schedule the units as
            # LATE as allowed: the endgame iterations (>=42) have no A/B
            # units left (all deadlines passed) and otherwise starve PE.
            DSLOT = {0: (15, 17, 19, 21, 23, 25),
                     1: (27, 30, 33, 36, 39, 42),
                     2: (43, 44, 45, 46, 47, 47)}
            DSLOT = {0: (15, 17, 19, 21, 23, 25),
                     1: (27, 30, 34, 38, 42, 44),
                     2: (44, 45, 46, 46, 47, 47)}
            dsched = {}
            for nt in range(NT - 1):
                for mt in range(KT):
                    dsched.setdefault(DSLOT[nt][mt], []).append(
                        lambda nt=nt, mt=mt: unit_d(nt, mt)
                    )

            # ---- prologue ----
            for hp in range(KT):
                (nc.scalar if hp % 2 == 0 else nc.gpsimd).dma_start(
                    out=bias_sb[hp][:], in_=bT[hp]
                )
            nc.sync.dma_start(out=wpj8_sb[:, 0], in_=wpj8[0])
            nc.sync.dma_start(out=wpj8_sb[:, 1], in_=wpj8[1])
            # prologue B units run 2-per-psC-tile (6 banks idle here), so
            # PE isn't paced by the psA double-buffer eviction latency
            for b in range(2):
                for jt in range(2):
                    t_ = psC.tile([128, 2, 512], f32, tag="psC")
                    for n2 in range(2):
                        unit_b(b, jt, n2, psv=t_[:, n2])

            unit_a(0, 0)
            unit_a(KT, 0)

            # ---- flat pipelined loop ----
            NITER = KT * BPC
            TARGET = 1250  # ns of PE filler per iteration
            ABCOST = 700   # avg A/B unit PE ns
            ai = 0
            for g in range(NITER + 1):
                # hard-due AB units must precede S[g]
                spent = 0
                while ai < len(abunits) and abunits[ai][0] <= g:
                    abunits[ai][1]()
                    ai += 1
                    spent += ABCOST
                if g < NITER:
                    e2 = wp.tile([128, 2, 2, N], bf16, tag="e2", bufs=3)
                    s_tiles[g] = e2
                    emit_s(g, e2)
                # scheduled D units + EDF top-up between S[g] and AV[g-1]
                for fn in dsched.get(g, ()):
                    fn()
                    spent += 738
                while spent < TARGET and ai < len(abunits):
                    abunits[ai][1]()
                    ai += 1
                    spent += ABCOST
                if g == NITER:
                    # last window: pair p=0,1 partials only need head-pairs
                    # 0..3; they fill PE while the final AV/normalize chain
                    # completes.  Two extra 2-bank psC tiles host one output
                    # tile's group per bank.
                    d3 = []
                    for mt in range(2):
                        t_ = psA.tile([128, 512], f32, tag="psA")
                        d3.append(t_[:])
                        unit_d_mm(t_[:], NT - 1, mt, (0, 1))
                if g >= 1:
                    emit_av(g - 1)
            for mt in range(2):
                unit_d_mm(d3[mt], NT - 1, mt, (2,))
                unit_d_fin(d3[mt], NT - 1, mt, act=True)
            for mt in range(2, KT):
                ps = psA.tile([128, 512], f32, tag="psA")
                unit_d_mm(ps, NT - 1, mt, (0, 1, 2))
                unit_d_fin(ps, NT - 1, mt, act=True)

    if finalize:
        nc.finalize()
    return nc


def _split8(a):
    """Error-compensated fp8 pair: a ~= hi + lo, each e4m3."""
    hi = a.astype(F8)
    lo = (a - hi.astype(np.float32)).astype(F8)
    return hi, lo


def _ktiles(a, nf):
    """(768, nf) -> (128, KT, nf) partition-major k-tiles."""
    return np.ascontiguousarray(a.reshape(KT, 128, nf).transpose(1, 0, 2))


def _host_prep(x, qkv_w, qkv_b, proj_w, proj_b, rel_table, log_temp, rel_index):
    """Build the per-core input maps (host-side layout prep only)."""
    x = np.asarray(x, np.float32)
    qkv_w = np.asarray(qkv_w, np.float32)
    qkv_b = np.asarray(qkv_b, np.float32)
    proj_w = np.asarray(proj_w, np.float32)
    rel_table = np.asarray(rel_table, np.float32)
    log_temp = np.asarray(log_temp, np.float32)
    rel_index = np.asarray(rel_index)

    temp = np.log1p(np.exp(log_temp.astype(np.float64))).astype(np.float32)
    alpha = (SCALE / temp).astype(np.float32)         # (H,) folded into q
    alpha_c = np.repeat(alpha, D)                     # (768,)

    # qk weights, host-scaled for fp8 range (SQ incl. alpha; SK plain),
    # split into hi/lo e4m3 pairs; hi-only and interleaved-cross layouts
    wqkT = qkv_w[0 : 2 * C].T.copy()                  # (768, 1536)
    wqkT[:, 0:C] *= alpha_c[None, :] * SQ
    wqkT[:, C : 2 * C] *= SK
    qhi, qlo = _split8(wqkT)
    qhi_t = _ktiles(qhi.astype(np.float32), 2 * C)
    qlo_t = _ktiles(qlo.astype(np.float32), 2 * C)
    wqk_hi_np = np.ascontiguousarray(
        qhi_t.reshape(128, KT // 2, 2, 2 * KT, 128).transpose(0, 3, 1, 2, 4)
    ).astype(F8)
    # cross weights LO-FIRST so the shared x cross buffer can stay HI-FIRST
    wqk_x_np = np.ascontiguousarray(
        np.stack([qlo_t, qhi_t], axis=2)
        .reshape(128, KT, 2, 2 * KT, 128)
        .transpose(0, 3, 1, 2, 4)
    ).astype(F8)

    # wv as fp8 hi/lo cross pairs, LO-FIRST (moving operand of V phase),
    # host-scaled by SVF out of the e4m3 subnormal range
    wvT = qkv_w[2 * C : 3 * C].T * SVF                # (768, 768)
    vhi, vlo = _split8(wvT)
    vhi_t = _ktiles(vhi.astype(np.float32), C)
    vlo_t = _ktiles(vlo.astype(np.float32), C)
    wvx = np.stack([vlo_t, vhi_t], axis=2)            # (128, KT, 2, C) lo-first
    wv_x_np = np.stack(
        [wvx[:, :, :, 0 : C // 2], wvx[:, :, :, C // 2 : C]], axis=0
    ).astype(F8)
    wpjT = proj_w.T * SP                              # (768, 768)
    phi, plo = _split8(wpjT)
    phi_t = _ktiles(phi.astype(np.float32), C).reshape(128, KT // 2, 2, C)
    plo_t = _ktiles(plo.astype(np.float32), C).reshape(128, KT // 2, 2, C)
    wpj8_np = np.stack(
        [
            np.ascontiguousarray(
                t.reshape(128, KT // 2, 2, KT, 128).transpose(0, 3, 1, 2, 4)
            )
            for t in (phi_t, plo_t)
        ],
        axis=0,
    ).astype(F8)

    bq = qkv_b[0:C] * alpha_c
    bk = qkv_b[C : 2 * C]
    bqk_np = np.concatenate([bq, bk]).reshape(2 * KT, 128).T.copy().astype(np.float32)

    # multiplicative bias table: exp((relpos bias)/temp), diag -> 0, CLS -> 1,
    # transposed to (j, i); paired layout (KT, j, 2N)
    rpb = rel_table[rel_index]                        # (196, 196, H)
    bias = np.zeros((H, N, N), np.float32)
    bias[:, 1:, 1:] = rpb.transpose(2, 0, 1) / temp[:, None, None]
    ebias = np.exp(bias)
    idx = np.arange(1, N)
    ebias[:, idx, idx] = 0.0
    ebT = ebias.transpose(0, 2, 1)                    # (H, j, i)
    bT_np = np.zeros((KT, 128, 4, N), np.float32)
    for jt, rows in enumerate(JROWS):
        blk = ebT[:, jt * 128 : jt * 128 + rows, :]   # (H, rows, N)
        bT_np[:, 0:rows, 2 * jt : 2 * jt + 2] = (
            blk.reshape(KT, 2, rows, N).transpose(0, 2, 1, 3)
        )
    bT_np = bT_np.astype(BF16)


    in_maps = []
    for c in range(NCORES):
        xc = x[c * BPC : (c + 1) * BPC].reshape(T, C).T  # (768, T)
        xhi, xlo = _split8(xc)
        xhi_t = _ktiles(xhi.astype(np.float32), T)      # (128, KT, T)
        xlo_t = _ktiles(xlo.astype(np.float32), T)
        x_hi_np = np.zeros((128, KT // 2, 2, TQ), np.float32)
        x_hi_np[:, :, :, 0:T] = xhi_t.reshape(128, KT // 2, 2, T)
        x_hi_np = x_hi_np.astype(F8)
        x_x_np = np.zeros((128, KT, 2, TQ), np.float32)
        x_x_np[:, :, :, 0:T] = np.stack([xhi_t, xlo_t], axis=2)
        x_x_np = x_x_np.astype(F8)
        in_maps.append(
            {
                "x_hi": x_hi_np,
                "x_x": x_x_np,
                "wv_x": wv_x_np,
                "wqk_hi": wqk_hi_np,
                "wqk_x": wqk_x_np,
                "wpj8": wpj8_np,
                "bT": bT_np,
                "bqk": bqk_np,
            }
        )
    return in_maps


def kernel(**inputs) -> np.ndarray:
    global LAST_RESULTS
    from concourse.bass_utils import run_bass_kernel_spmd

    if "nc" not in _CACHE:
        _CACHE["nc"] = _build()
    nc = _CACHE["nc"]

    in_maps = _host_prep(**inputs)
    try:
        res = run_bass_kernel_spmd(
            nc, in_maps, core_ids=list(range(NCORES)), trace=TRACE
        )
    except ModuleNotFoundError:
        res = run_bass_kernel_spmd(
            nc, in_maps, core_ids=list(range(NCORES)), trace=False
        )
    LAST_RESULTS = res

    # v-bias rides through attention unchanged (rows of attn sum to 1), so
    # its proj image folds into the constant output bias added here
    proj_b = np.asarray(inputs["proj_b"], np.float32)
    proj_w = np.asarray(inputs["proj_w"], np.float32)
    bv = np.asarray(inputs["qkv_b"], np.float32)[2 * C : 3 * C]
    b_eff = proj_b + proj_w @ bv
    outs = []
    for c in range(NCORES):
        oT = np.asarray(res.results[c]["outT"], np.float32).reshape(C, T)
        outs.append(oT.T.reshape(BPC, N, C))
    out = np.concatenate(outs, axis=0) + b_eff[None, None, :]
    return out.astype(np.float32)


# revision 36
# speedup vs baseline: 1.0048x; 1.0028x over previous
"""Trainium2 Bass kernel for nn_Attention_11055245820093.

Swin-style attention block: qkv proj -> per-head scaled dot-product attention
with 2D relative position bias (CLS zero-padded), per-head softplus temperature,
patch-diagonal mask -> proj.

Strategy: data-parallel over batch B=64 across 8 NeuronCores (8 batches/core).
All compute per core runs in a "transposed" layout (channels on partitions,
tokens on the free dim) so no on-device transposes are needed.

Numerics (measured rel_err 1.55e-2 vs the 2e-2 gate; inputs are
deterministic):
  - QK/V projections in fp8e4m3 DoubleRow: W_hi*x_hi pair terms plus
    interleaved cross terms (W_lo*x_hi + W_hi*x_lo per k-tile).  The QK
    proj drops the cross terms of k-tiles {0, 3} -- each dropped k-tile
    trades ~7e-3 of (quadrature) error for 3.9us of PE time.
  - Attention (QK^T, exp, bias, AV, softmax divide) in bf16: any
    UNcompensated e4m3 activation quantization alone costs ~2.5e-2
    max-rel, so fp8 attention is not affordable.
  - Output proj in fp8 DoubleRow with BOTH sides compensated: wpj hi/lo
    pairs (host) x attn hi/lo pairs (split on GPSIMD after the softmax
    divide; attn is scaled by 64 via the V-path so fp8 is out of the
    subnormal range).  9 DR instructions = 4/3x the bf16 rate.
  - Output staged/DMA'd in bf16 (adds ~2e-3), final f32 convert on host.

Performance structure (112.8us total, PE busy 100us at 88.6%):
  - ONE flat software-pipelined loop over (batch, head-pair): iteration g
    issues S[g] (QK^T), then PE "filler" units, then AV[g-1] -- AV runs one
    iteration behind so its exp/bias-mul chain (ACT+Pool) has a full
    iteration of slack.
  - Batch-major order completes attention token-windows early, so
    output-proj units become mid-loop PE filler instead of a tail burst.
  - PE filler units follow a static earliest-deadline schedule: V-proj
    units B[b] (due before AV[b]), QK-proj window subtiles (due before the
    first S that reads them; K windows due one batch-pair early), and
    output-proj groups D(nt, mt) (scheduled as LATE as allowed -- the
    endgame iterations have no other filler).  The last window's D groups
    emit their first 6 DR terms before the final AV so the drain stays
    PE-busy.
  - x is stored once, flat (channel-pair-major, token columns padded to
    1584 for the 16B DoubleRow stride rule); V-proj takes arbitrary
    token-window slices of it as the stationary operand, QK-proj takes
    394-token moving slices (9 matmuls per subtile).
  - bias multiply (e = exp(S) * ebias) and the attn hi/lo fp8 split run
    on GPSIMD/Pool (proxy library tensor_tensor, SBUF-only); softmax
    reciprocal+broadcast run per-head so DVE and Pool pipeline; evictions
    alternate ACT/DVE 5:3.
  - PSUM: 2-bank unit pool (double-buffered) + 3x2-bank attention pool
    (S-jt0, S-jt1, AV+denominator rotate); prologue V-proj units borrow
    the idle attention banks to run 6-deep.
  - big DMAs split fine-grained across the SP/ACT/Pool queues in
    consumption order (1.7us init latency + 500ns min per transfer).
"""

import os
import sys

sys.path.insert(0, "/opt/trn_rl_repo")
os.environ.setdefault("MYCRO_LOCAL_CACHE", "1")

import numpy as np
import ml_dtypes

BF16 = ml_dtypes.bfloat16
F8 = ml_dtypes.float8_e4m3fn

# Problem constants (hardcoded per contract)
B, N, C, H, D = 64, 197, 768, 12, 64
NCORES = 8
BPC = B // NCORES          # 8 batches per core
T = BPC * N                # 1576 tokens per core
KT = C // 128              # 6 contraction tiles of 128
NT = 4                     # token n-tiles
TN = T // NT               # 394 tokens per n-tile
SCALE = D ** -0.5
JROWS = (128, N - 128)     # 128, 69
N2 = 2 * N
SVF = 64.0                 # wv host-scale; ones column matches so the
                           # softmax divide cancels it exactly
SQ, SK, SP = 256.0, 64.0, 64.0
TQ = 1584                  # flat x / attn pad (pair step 16-aligned)

_CACHE = {}

TRACE = False
LAST_RESULTS = None


def _build(finalize=True):
    import concourse.bass as bass
    import concourse.tile as tile
    from concourse import bacc, library_config, mybir

    dt = mybir.dt
    f32, bf16, f8 = dt.float32, dt.bfloat16, dt.float8e4
    AF = mybir.ActivationFunctionType
    OP = mybir.AluOpType
    DR = mybir.MatmulPerfMode.DoubleRow

    nc = bacc.Bacc("TRN2", target_bir_lowering=False, debug=False)

    x_hi = nc.dram_tensor(
        "x_hi", [128, KT // 2, 2, TQ], f8, kind="ExternalInput"
    ).ap()
    x_x = nc.dram_tensor(
        "x_x", [128, KT, 2, TQ], f8, kind="ExternalInput"
    ).ap()
    wv_x = nc.dram_tensor(
        "wv_x", [2, 128, KT, 2, C // 2], f8, kind="ExternalInput"
    ).ap()
    wqk_hi = nc.dram_tensor(
        "wqk_hi", [128, 2 * KT, KT // 2, 2, 128], f8, kind="ExternalInput"
    ).ap()
    wqk_x = nc.dram_tensor(
        "wqk_x", [128, 2 * KT, KT, 2, 128], f8, kind="ExternalInput"
    ).ap()
    wpj8 = nc.dram_tensor(
        "wpj8", [2, 128, KT, KT // 2, 2, 128], f8, kind="ExternalInput"
    ).ap()
    bT = nc.dram_tensor("bT", [KT, 128, 4, N], bf16, kind="ExternalInput").ap()
    bqk = nc.dram_tensor("bqk", [128, 2 * KT], f32, kind="ExternalInput").ap()
    outT = nc.dram_tensor("outT", [KT, 128, T], bf16, kind="ExternalOutput").ap()


    with tile.TileContext(nc) as tc:
        from contextlib import ExitStack

        with ExitStack() as ctx:
            nc.gpsimd.load_library(library_config.proxy)
            cp = ctx.enter_context(tc.tile_pool(name="consts", bufs=1))
            psA = ctx.enter_context(tc.tile_pool(name="psA", bufs=2, space="PSUM"))
            psC = ctx.enter_context(tc.tile_pool(name="psC", bufs=3, space="PSUM"))
            wp = ctx.enter_context(tc.tile_pool(name="work", bufs=2))

            # ---- persistent SBUF tiles; DMAs in consumption order ----
            xh_sb = cp.tile([128, KT // 2, 2, TQ], f8, name="xh", tag="xh")
            xx_sb = cp.tile([128, KT, 2, TQ], f8, name="xx", tag="xx")
            wvx_sb = cp.tile([128, 2, KT, 2, C // 2], f8, name="wvx", tag="wvx")
            wqkh_sb = cp.tile(
                [128, 2 * KT, KT // 2, 2, 128], f8, name="wqkh", tag="wqkh"
            )
            wqkx_sb = cp.tile(
                [128, 2 * KT, KT, 2, 128], f8, name="wqkx", tag="wqkx"
            )
            bqk_sb = cp.tile([128, 2 * KT], f32, name="bqk", tag="bqk")
            # startup-critical DMAs, spread so each consumer unblocks at
            # its need time: B prologue needs xh0/xx0/wvx; prologue A units
            # need only the mt=0 and mt=6 slices of wqk; later windows and
            # head-pairs stream in during the loop
            nc.gpsimd.dma_start(out=wvx_sb[:, 0], in_=wv_x[0])
            nc.gpsimd.dma_start(out=wvx_sb[:, 1], in_=wv_x[1])
            # batch-0 token span lands first, then batch 1, then the rest
            nc.scalar.dma_start(out=xh_sb[:, :, :, 0:N], in_=x_hi[:, :, :, 0:N])
            nc.sync.dma_start(out=xx_sb[:, :, :, 0:N], in_=x_x[:, :, :, 0:N])
            nc.scalar.dma_start(out=xh_sb[:, :, :, N:TN], in_=x_hi[:, :, :, N:TN])
            nc.sync.dma_start(out=xx_sb[:, :, :, N:TN], in_=x_x[:, :, :, N:TN])
            # wqk mt-slices in hp-need order (both Q and K of pair hp are
            # needed by iteration hp), alternating gpsimd/sync
            for hp in range(KT):
                qa = nc.gpsimd if hp % 2 == 0 else nc.sync
                for m in (hp, KT + hp):
                    qa.dma_start(out=wqkh_sb[:, m], in_=wqk_hi[:, m])
                    qa.dma_start(out=wqkx_sb[:, m], in_=wqk_x[:, m])
            nc.scalar.dma_start(out=bqk_sb[:], in_=bqk[:])
            for nt in range(1, NT):
                o = nt * TN
                e_ = TQ if nt == NT - 1 else (nt + 1) * TN
                nc.sync.dma_start(out=xh_sb[:, :, :, o:e_], in_=x_hi[:, :, :, o:e_])
                nc.sync.dma_start(out=xx_sb[:, :, :, o:e_], in_=x_x[:, :, :, o:e_])
            bias_sb = [
                cp.tile([128, 4, N], bf16, name=f"bias{hp}", tag=f"bias{hp}")
                for hp in range(KT)
            ]
            wpj8_sb = cp.tile(
                [128, 2, KT, KT // 2, 2, 128], f8, name="wpj8", tag="wpj8"
            )
            TP = T + 8  # attn pair-step must be 16-byte aligned
            attnp = [
                cp.tile([128, KT // 2, 2, TP], f8, name=f"atp{kind}", tag=f"atp{kind}")
                for kind in range(2)
            ]

            # qk tiles: Q (mt 0..5), K (mt 6..11)
            qk_sb = [
                cp.tile([128, T], bf16, name=f"qk{m}", tag=f"qk{m}")
                for m in range(2 * KT)
            ]
            # V per (batch, jt): (rows, 12 heads, 65) -- 64 V cols + ones col
            v_sb = {}
            for b in range(BPC):
                for jt, rows in enumerate(JROWS):
                    t_ = cp.tile(
                        [rows, H, D + 1], bf16, name=f"v{b}_{jt}", tag=f"v{b}_{jt}"
                    )
                    nc.vector.memset(t_[:, :, D : D + 1], 1.0)
                    v_sb[(b, jt)] = t_

            evict_flip = [0]

            def evict_engine():
                evict_flip[0] += 1
                return nc.vector if evict_flip[0] % 8 in (0, 3, 6) else nc.scalar

            # ---- filler unit emitters (pure PE work + one eviction) ----
            def unit_b(b, jt, n2, psv=None):
                """V-proj quarter: one psum group -> v_sb[(b, jt)] slice."""
                rows = JROWS[jt]
                o = b * N + jt * 128
                if psv is None:
                    psv = psA.tile([128, 512], f32, tag="psA")
                for p in range(KT // 2):
                    nc.tensor.matmul(
                        psv[0:rows, 0 : C // 2],
                        xh_sb[:, p, :, o : o + rows],
                        wvx_sb[:, n2, 2 * p : 2 * p + 2, 1, :],
                        start=(p == 0),
                        stop=False,
                        perf_mode=DR,
                    )
                for k in range(KT):
                    nc.tensor.matmul(
                        psv[0:rows, 0 : C // 2],
                        xx_sb[:, k, :, o : o + rows],
                        wvx_sb[:, n2, k, :, :],
                        start=False,
                        stop=(k == KT - 1),
                        perf_mode=DR,
                    )
                eng = evict_engine()
                dst = v_sb[(b, jt)][0:rows, n2 * KT : (n2 + 1) * KT, 0:D]
                src = psv[0:rows, 0 : C // 2].rearrange("p (h d) -> p h d", h=KT)
                if eng is nc.vector:
                    nc.vector.tensor_copy(dst, src)
                else:
                    nc.scalar.activation(dst, src, AF.Copy)

            # cross terms for these k-tiles are dropped in the QK proj
            # (error-compensation budget spent for PE time; validated at
            # ~1.5e-2 total rel err vs the 2e-2 gate)
            ADROP = (0, 3)
            AKEEP = tuple(k for k in range(KT) if k not in ADROP)

            def unit_a(mt, nt, eng=None):
                """QK-proj subtile: one token window of Q or K tile mt."""
                inv_s = (1.0 / SQ) if mt < KT else (1.0 / SK)
                o = nt * TN
                ps = psA.tile([128, 512], f32, tag="psA")
                for p in range(KT // 2):
                    nc.tensor.matmul(
                        ps[:, 0:TN],
                        wqkh_sb[:, mt, p, :, :],
                        xh_sb[:, p, :, o : o + TN],
                        start=(p == 0),
                        stop=False,
                        perf_mode=DR,
                    )
                for k in AKEEP:
                    nc.tensor.matmul(
                        ps[:, 0:TN],
                        wqkx_sb[:, mt, k, :, :],
                        xx_sb[:, k, :, o : o + TN],
                        start=False,
                        stop=(k == AKEEP[-1]),
                        perf_mode=DR,
                    )
                dst = qk_sb[mt][:, nt * TN : (nt + 1) * TN]
                srcp = ps[:, 0:TN]
                if (eng or evict_engine()) is nc.vector:
                    nc.vector.tensor_scalar(
                        dst, srcp, inv_s, bqk_sb[:, mt : mt + 1],
                        OP.mult, OP.add,
                    )
                else:
                    nc.scalar.activation(
                        dst, srcp, AF.Identity,
                        bias=bqk_sb[:, mt : mt + 1], scale=inv_s,
                    )

            dq = [0]

            DTERMS = [(0, 0), (1, 0), (0, 1)]  # (weight kind, attn kind)

            def unit_d_mm(ps, nt, mt, ps_):
                """ps_ = list of pair indices p to accumulate (0..2)."""
                for ti, (wk, ak) in enumerate(DTERMS):
                    for p in ps_:
                        nc.tensor.matmul(
                            ps[:, 0:TN],
                            wpj8_sb[:, wk, mt, p, :, :],
                            attnp[ak][:, p, :, nt * TN : (nt + 1) * TN],
                            start=(ti == 0 and p == 0),
                            stop=(ti == 2 and p == 2),
                            perf_mode=DR,
                        )

            def unit_d_fin(ps, nt, mt, act=None):
                ot = wp.tile([128, TN], bf16, tag="ot", bufs=3)
                use_dve = (act is False) if act is not None else (
                    evict_engine() is nc.vector)
                if use_dve:
                    nc.vector.tensor_scalar_mul(ot[:], ps[:, 0:TN], 1.0 / 4096.0)
                else:
                    nc.scalar.activation(ot[:], ps[:, 0:TN], AF.Copy,
                                         scale=1.0 / 4096.0)
                dq[0] += 1
                nc.sync.dma_start(
                    out=outT[mt, :, nt * TN : (nt + 1) * TN], in_=ot[:]
                )

            def unit_d(nt, mt):
                """output-proj group: one (window, out-tile) -> outT DMA."""
                ps = psA.tile([128, 512], f32, tag="psA")
                unit_d_mm(ps, nt, mt, (0, 1, 2))
                unit_d_fin(ps, nt, mt)

            # ---- attention pieces ----
            s_tiles = {}

            def emit_s(g, e2):
                """S^T matmuls + exp + Pool bias-mul, per key-window."""
                b, hp = divmod(g, KT)
                for jt, rows in enumerate(JROWS):
                    ps = psC.tile([128, 2, 512], f32, tag="psC")
                    for hh in range(2):
                        base = 64 * hh
                        i0 = b * N + jt * 128
                        nc.tensor.matmul(
                            ps[0:rows, hh, 0:N],
                            qk_sb[KT + hp][base : base + 64, i0 : i0 + rows],
                            qk_sb[hp][base : base + 64, b * N : (b + 1) * N],
                            start=True,
                            stop=True,
                        )
                    eu = wp.tile([128, 2, N], bf16, tag=f"eu{jt}", bufs=3)
                    nc.scalar.activation(
                        eu[0:rows, :, :], ps[0:rows, :, 0:N], AF.Exp
                    )
                    nc.gpsimd.tensor_mul(
                        e2[0:rows, :, jt, :],
                        eu[0:rows, :, :],
                        bias_sb[hp][0:rows, 2 * jt : 2 * jt + 2, :],
                    )

            def emit_av(g):
                """AV + softmax normalize for iteration g (runs at g+1)."""
                b, hp = divmod(g, KT)
                e2 = s_tiles.pop(g)
                po = psC.tile([128, 2, 512], f32, tag="psC")
                for hh in range(2):
                    h = 2 * hp + hh
                    for jt, rows in enumerate(JROWS):
                        nc.tensor.matmul(
                            po[0 : D + 1, hh, 0:N],
                            v_sb[(b, jt)][0:rows, h, 0 : D + 1],
                            e2[0:rows, hh, jt, :],
                            start=(jt == 0),
                            stop=(jt == 1),
                        )
                # per-head recip/bcast/mul chains pipeline DVE and Pool,
                # shortening the latency until po's PSUM banks free up
                r2 = wp.tile([1, 2, N], bf16, tag="r2", bufs=3)
                rb = wp.tile([128, N2], bf16, tag="rb", bufs=3)
                at = wp.tile([128, N], bf16, tag="atmp", bufs=3)
                with nc.allow_low_precision(
                    reason="softmax denom reciprocal in bf16"
                ):
                    nc.vector.reciprocal(r2[:, 0, :], po[D : D + 1, 0, 0:N])
                    nc.gpsimd.partition_broadcast(rb[:, 0:N], r2[:, 0, :])
                    nc.vector.reciprocal(r2[:, 1, :], po[D : D + 1, 1, 0:N])
                    nc.gpsimd.partition_broadcast(rb[:, N:N2], r2[:, 1, :])
                nc.vector.tensor_mul(at[0:D, :], po[0:D, 0, 0:N], rb[0:D, 0:N])
                nc.vector.tensor_mul(
                    at[D : 2 * D, :], po[0:D, 1, 0:N], rb[D : 2 * D, N:N2]
                )
                # fp8 hi/lo split of attn^T (feeds the DR output proj)
                nc.gpsimd.tensor_copy(
                    attnp[0][:, hp // 2, hp % 2, b * N : (b + 1) * N], at[:]
                )
                nc.gpsimd.tensor_sub(
                    attnp[1][:, hp // 2, hp % 2, b * N : (b + 1) * N],
                    at[:],
                    attnp[0][:, hp // 2, hp % 2, b * N : (b + 1) * N],
                )

            # ---- static filler schedule ----
            # AB units have HARD deadlines (due = first iteration whose S or
            # AV reads their output; emitting later would cycle the in-order
            # PE queue through an ACT/DVE eviction that sits behind stalled
            # work -> deadlock).  D units instead have a READY iteration
            # (earliest emission keeping attn writes ahead in PE order).
            abunits = []
            for b in range(2, BPC):
                for jt in range(2):
                    for n2 in range(2):
                        # V[b] consumed by AV[b, hp0] emitted at iter 6b+1
                        abunits.append(
                            (max(0, KT * b - 1),
                             lambda b=b, jt=jt, n2=n2: unit_b(b, jt, n2))
                        )
            for hp in range(KT):
                for nt in range(NT):
                    for mt in (hp, KT + hp):
                        if hp == 0 and nt == 0:
                            continue  # prologue
                        # first S reading window nt of pair hp: b = 2nt
                        abunits.append(
                            (2 * nt * KT + hp,
                             lambda mt=mt, nt=nt: unit_a(mt, nt))
                        )
            abunits.sort(key=lambda u: u[0])
            # D(nt) is ready from iteration 12nt+13 (window nt's last norms
            # are emitted during iteration 12nt+12).  Schedule the units as
            # LATE as allowed: the endgame iterations (>=42) have no A/B
            # units left (all deadlines passed) and otherwise starve PE.
            DSLOT = {0: (15, 17, 19, 21, 23, 25),
                     1: (27, 30, 33, 36, 39, 42),
                     2: (43, 44, 45, 46, 47, 47)}
            DSLOT = {0: (15, 17, 19, 21, 23, 25),
                     1: (27, 30, 34, 38, 42, 44),
                     2: (44, 45, 46, 46, 47, 47)}
            dsched = {}
            for nt in range(NT - 1):
                for mt in range(KT):
                    dsched.setdefault(DSLOT[nt][mt], []).append(
                        lambda nt=nt, mt=mt: unit_d(nt, mt)
                    )

            # ---- prologue ----
            for hp in range(KT):
                (nc.scalar if hp % 2 == 0 else nc.gpsimd).dma_start(
                    out=bias_sb[hp][:], in_=bT[hp]
                )
            nc.sync.dma_start(out=wpj8_sb[:, 0], in_=wpj8[0])
            nc.sync.dma_start(out=wpj8_sb[:, 1], in_=wpj8[1])
            # prologue B units run 2-per-psC-tile (6 banks idle here), so
            # PE isn't paced by the psA double-buffer eviction latency
            for b in range(2):
                for jt in range(2):
                    t_ = psC.tile([128, 2, 512], f32, tag="psC")
                    for n2 in range(2):
                        unit_b(b, jt, n2, psv=t_[:, n2])

            unit_a(0, 0)
            unit_a(KT, 0)

            # ---- flat pipelined loop ----
            NITER = KT * BPC
            TARGET = 1400  # ns of PE filler per iteration
            ABCOST = 700   # avg A/B unit PE ns
            ai = 0
            for g in range(NITER + 1):
                # hard-due AB units must precede S[g]
                spent = 0
                while ai < len(abunits) and abunits[ai][0] <= g:
                    abunits[ai][1]()
                    ai += 1
                    spent += ABCOST
                if g < NITER:
                    e2 = wp.tile([128, 2, 2, N], bf16, tag="e2", bufs=3)
                    s_tiles[g] = e2
                    emit_s(g, e2)
                # scheduled D units + EDF top-up between S[g] and AV[g-1]
                for fn in dsched.get(g, ()):
                    fn()
                    spent += 738
                while spent < TARGET and ai < len(abunits):
                    abunits[ai][1]()
                    ai += 1
                    spent += ABCOST
                if g == NITER:
                    # last window: pair p=0,1 partials only need head-pairs
                    # 0..3; they fill PE while the final AV/normalize chain
                    # completes.  Two extra 2-bank psC tiles host one output
                    # tile's group per bank.
                    d3 = []
                    for mt in range(2):
                        t_ = psA.tile([128, 512], f32, tag="psA")
                        d3.append(t_[:])
                        unit_d_mm(t_[:], NT - 1, mt, (0, 1))
                if g >= 1:
                    emit_av(g - 1)
            for mt in range(2):
                unit_d_mm(d3[mt], NT - 1, mt, (2,))
                unit_d_fin(d3[mt], NT - 1, mt, act=True)
            for mt in range(2, KT):
                ps = psA.tile([128, 512], f32, tag="psA")
                unit_d_mm(ps, NT - 1, mt, (0, 1, 2))
                unit_d_fin(ps, NT - 1, mt, act=True)

    if finalize:
        nc.finalize()
    return nc


def _split8(a):
    """Error-compensated fp8 pair: a ~= hi + lo, each e4m3."""
    hi = a.astype(F8)
    lo = (a - hi.astype(np.float32)).astype(F8)
    return hi, lo


def _ktiles(a, nf):
    """(768, nf) -> (128, KT, nf) partition-major k-tiles."""
    return np.ascontiguousarray(a.reshape(KT, 128, nf).transpose(1, 0, 2))


def _host_prep(x, qkv_w, qkv_b, proj_w, proj_b, rel_table, log_temp, rel_index):
    """Build the per-core input maps (host-side layout prep only)."""
    x = np.asarray(x, np.float32)
    qkv_w = np.asarray(qkv_w, np.float32)
    qkv_b = np.asarray(qkv_b, np.float32)
    proj_w = np.asarray(proj_w, np.float32)
    rel_table = np.asarray(rel_table, np.float32)
    log_temp = np.asarray(log_temp, np.float32)
    rel_index = np.asarray(rel_index)

    temp = np.log1p(np.exp(log_temp.astype(np.float64))).astype(np.float32)
    alpha = (SCALE / temp).astype(np.float32)         # (H,) folded into q
    alpha_c = np.repeat(alpha, D)                     # (768,)

    # qk weights, host-scaled for fp8 range (SQ incl. alpha; SK plain),
    # split into hi/lo e4m3 pairs; hi-only and interleaved-cross layouts
    wqkT = qkv_w[0 : 2 * C].T.copy()                  # (768, 1536)
    wqkT[:, 0:C] *= alpha_c[None, :] * SQ
    wqkT[:, C : 2 * C] *= SK
    qhi, qlo = _split8(wqkT)
    qhi_t = _ktiles(qhi.astype(np.float32), 2 * C)
    qlo_t = _ktiles(qlo.astype(np.float32), 2 * C)
    wqk_hi_np = np.ascontiguousarray(
        qhi_t.reshape(128, KT // 2, 2, 2 * KT, 128).transpose(0, 3, 1, 2, 4)
    ).astype(F8)
    # cross weights LO-FIRST so the shared x cross buffer can stay HI-FIRST
    wqk_x_np = np.ascontiguousarray(
        np.stack([qlo_t, qhi_t], axis=2)
        .reshape(128, KT, 2, 2 * KT, 128)
        .transpose(0, 3, 1, 2, 4)
    ).astype(F8)

    # wv as fp8 hi/lo cross pairs, LO-FIRST (moving operand of V phase),
    # host-scaled by SVF out of the e4m3 subnormal range
    wvT = qkv_w[2 * C : 3 * C].T * SVF                # (768, 768)
    vhi, vlo = _split8(wvT)
    vhi_t = _ktiles(vhi.astype(np.float32), C)
    vlo_t = _ktiles(vlo.astype(np.float32), C)
    wvx = np.stack([vlo_t, vhi_t], axis=2)            # (128, KT, 2, C) lo-first
    wv_x_np = np.stack(
        [wvx[:, :, :, 0 : C // 2], wvx[:, :, :, C // 2 : C]], axis=0
    ).astype(F8)
    wpjT = proj_w.T * SP                              # (768, 768)
    phi, plo = _split8(wpjT)
    phi_t = _ktiles(phi.astype(np.float32), C).reshape(128, KT // 2, 2, C)
    plo_t = _ktiles(plo.astype(np.float32), C).reshape(128, KT // 2, 2, C)
    wpj8_np = np.stack(
        [
            np.ascontiguousarray(
                t.reshape(128, KT // 2, 2, KT, 128).transpose(0, 3, 1, 2, 4)
            )
            for t in (phi_t, plo_t)
        ],
        axis=0,
    ).astype(F8)

    bq = qkv_b[0:C] * alpha_c
    bk = qkv_b[C : 2 * C]
    bqk_np = np.concatenate([bq, bk]).reshape(2 * KT, 128).T.copy().astype(np.float32)

    # multiplicative bias table: exp((relpos bias)/temp), diag -> 0, CLS -> 1,
    # transposed to (j, i); paired layout (KT, j, 2N)
    rpb = rel_table[rel_index]                        # (196, 196, H)
    bias = np.zeros((H, N, N), np.float32)
    bias[:, 1:, 1:] = rpb.transpose(2, 0, 1) / temp[:, None, None]
    ebias = np.exp(bias)
    idx = np.arange(1, N)
    ebias[:, idx, idx] = 0.0
    ebT = ebias.transpose(0, 2, 1)                    # (H, j, i)
    bT_np = np.zeros((KT, 128, 4, N), np.float32)
    for jt, rows in enumerate(JROWS):
        blk = ebT[:, jt * 128 : jt * 128 + rows, :]   # (H, rows, N)
        bT_np[:, 0:rows, 2 * jt : 2 * jt + 2] = (
            blk.reshape(KT, 2, rows, N).transpose(0, 2, 1, 3)
        )
    bT_np = bT_np.astype(BF16)


    in_maps = []
    for c in range(NCORES):
        xc = x[c * BPC : (c + 1) * BPC].reshape(T, C).T  # (768, T)
        xhi, xlo = _split8(xc)
        xhi_t = _ktiles(xhi.astype(np.float32), T)      # (128, KT, T)
        xlo_t = _ktiles(xlo.astype(np.float32), T)
        x_hi_np = np.zeros((128, KT // 2, 2, TQ), np.float32)
        x_hi_np[:, :, :, 0:T] = xhi_t.reshape(128, KT // 2, 2, T)
        x_hi_np = x_hi_np.astype(F8)
        x_x_np = np.zeros((128, KT, 2, TQ), np.float32)
        x_x_np[:, :, :, 0:T] = np.stack([xhi_t, xlo_t], axis=2)
        x_x_np = x_x_np.astype(F8)
        in_maps.append(
            {
                "x_hi": x_hi_np,
                "x_x": x_x_np,
                "wv_x": wv_x_np,
                "wqk_hi": wqk_hi_np,
                "wqk_x": wqk_x_np,
                "wpj8": wpj8_np,
                "bT": bT_np,
                "bqk": bqk_np,
            }
        )
    return in_maps


def kernel(**inputs) -> np.ndarray:
    global LAST_RESULTS
    from concourse.bass_utils import run_bass_kernel_spmd

    if "nc" not in _CACHE:
        _CACHE["nc"] = _build()
    nc = _CACHE["nc"]

    in_maps = _host_prep(**inputs)
    try:
        res = run_bass_kernel_spmd(
            nc, in_maps, core_ids=list(range(NCORES)), trace=TRACE
        )
    except ModuleNotFoundError:
        res = run_bass_kernel_spmd(
            nc, in_maps, core_ids=list(range(NCORES)), trace=False
        )
    LAST_RESULTS = res

    # v-bias rides through attention unchanged (rows of attn sum to 1), so
    # its proj image folds into the constant output bias added here
    proj_b = np.asarray(inputs["proj_b"], np.float32)
    proj_w = np.asarray(inputs["proj_w"], np.float32)
    bv = np.asarray(inputs["qkv_b"], np.float32)[2 * C : 3 * C]
    b_eff = proj_b + proj_w @ bv
    outs = []
    for c in range(NCORES):
        oT = np.asarray(res.results[c]["outT"], np.float32).reshape(C, T)
        outs.append(oT.T.reshape(BPC, N, C))
    out = np.concatenate(outs, axis=0) + b_eff[None, None, :]
    return out.astype(np.float32)


# revision 37
# speedup vs baseline: 1.0105x; 1.0057x over previous
"""Trainium2 Bass kernel for nn_Attention_11055245820093.

Swin-style attention block: qkv proj -> per-head scaled dot-product attention
with 2D relative position bias (CLS zero-padded), per-head softplus temperature,
patch-diagonal mask -> proj.

Strategy: data-parallel over batch B=64 across 8 NeuronCores (8 batches/core).
All compute per core runs in a "transposed" layout (channels on partitions,
tokens on the free dim) so no on-device transposes are needed.

Numerics (measured rel_err 1.55e-2 vs the 2e-2 gate; inputs are
deterministic):
  - QK/V projections in fp8e4m3 DoubleRow: W_hi*x_hi pair terms plus
    interleaved cross terms (W_lo*x_hi + W_hi*x_lo per k-tile).  The QK
    proj drops the cross terms of k-tiles {0, 3} -- each dropped k-tile
    trades ~7e-3 of (quadrature) error for 3.9us of PE time.
  - Attention (QK^T, exp, bias, AV, softmax divide) in bf16: any
    UNcompensated e4m3 activation quantization alone costs ~2.5e-2
    max-rel, so fp8 attention is not affordable.
  - Output proj in fp8 DoubleRow with BOTH sides compensated: wpj hi/lo
    pairs (host) x attn hi/lo pairs (split on GPSIMD after the softmax
    divide; attn is scaled by 64 via the V-path so fp8 is out of the
    subnormal range).  9 DR instructions = 4/3x the bf16 rate.
  - Output staged/DMA'd in bf16 (adds ~2e-3), final f32 convert on host.

Performance structure (112.8us total, PE busy 100us at 88.6%):
  - ONE flat software-pipelined loop over (batch, head-pair): iteration g
    issues S[g] (QK^T), then PE "filler" units, then AV[g-1] -- AV runs one
    iteration behind so its exp/bias-mul chain (ACT+Pool) has a full
    iteration of slack.
  - Batch-major order completes attention token-windows early, so
    output-proj units become mid-loop PE filler instead of a tail burst.
  - PE filler units follow a static earliest-deadline schedule: V-proj
    units B[b] (due before AV[b]), QK-proj window subtiles (due before the
    first S that reads them; K windows due one batch-pair early), and
    output-proj groups D(nt, mt) (scheduled as LATE as allowed -- the
    endgame iterations have no other filler).  The last window's D groups
    emit their first 6 DR terms before the final AV so the drain stays
    PE-busy.
  - x is stored once, flat (channel-pair-major, token columns padded to
    1584 for the 16B DoubleRow stride rule); V-proj takes arbitrary
    token-window slices of it as the stationary operand, QK-proj takes
    394-token moving slices (9 matmuls per subtile).
  - bias multiply (e = exp(S) * ebias) and the attn hi/lo fp8 split run
    on GPSIMD/Pool (proxy library tensor_tensor, SBUF-only); softmax
    reciprocal+broadcast run per-head so DVE and Pool pipeline; evictions
    alternate ACT/DVE 5:3.
  - PSUM: 2-bank unit pool (double-buffered) + 3x2-bank attention pool
    (S-jt0, S-jt1, AV+denominator rotate); prologue V-proj units borrow
    the idle attention banks to run 6-deep.
  - big DMAs split fine-grained across the SP/ACT/Pool queues in
    consumption order (1.7us init latency + 500ns min per transfer).
"""

import os
import sys

sys.path.insert(0, "/opt/trn_rl_repo")
os.environ.setdefault("MYCRO_LOCAL_CACHE", "1")

import numpy as np
import ml_dtypes

BF16 = ml_dtypes.bfloat16
F8 = ml_dtypes.float8_e4m3fn

# Problem constants (hardcoded per contract)
B, N, C, H, D = 64, 197, 768, 12, 64
NCORES = 8
BPC = B // NCORES          # 8 batches per core
T = BPC * N                # 1576 tokens per core
KT = C // 128              # 6 contraction tiles of 128
NT = 4                     # token n-tiles
TN = T // NT               # 394 tokens per n-tile
SCALE = D ** -0.5
JROWS = (128, N - 128)     # 128, 69
N2 = 2 * N
SVF = 64.0                 # wv host-scale; ones column matches so the
                           # softmax divide cancels it exactly
SQ, SK, SP = 256.0, 64.0, 64.0
TQ = 1584                  # flat x / attn pad (pair step 16-aligned)

_CACHE = {}

TRACE = False
LAST_RESULTS = None


def _build(finalize=True):
    import concourse.bass as bass
    import concourse.tile as tile
    from concourse import bacc, library_config, mybir

    dt = mybir.dt
    f32, bf16, f8 = dt.float32, dt.bfloat16, dt.float8e4
    AF = mybir.ActivationFunctionType
    OP = mybir.AluOpType
    DR = mybir.MatmulPerfMode.DoubleRow

    nc = bacc.Bacc("TRN2", target_bir_lowering=False, debug=False)

    x_hi = nc.dram_tensor(
        "x_hi", [128, KT // 2, 2, TQ], f8, kind="ExternalInput"
    ).ap()
    x_x = nc.dram_tensor(
        "x_x", [128, KT, 2, TQ], f8, kind="ExternalInput"
    ).ap()
    wv_x = nc.dram_tensor(
        "wv_x", [2, 128, KT, 2, C // 2], f8, kind="ExternalInput"
    ).ap()
    wqk_hi = nc.dram_tensor(
        "wqk_hi", [128, 2 * KT, KT // 2, 2, 128], f8, kind="ExternalInput"
    ).ap()
    wqk_x = nc.dram_tensor(
        "wqk_x", [128, 2 * KT, KT, 2, 128], f8, kind="ExternalInput"
    ).ap()
    wpj8 = nc.dram_tensor(
        "wpj8", [2, 128, KT, KT // 2, 2, 128], f8, kind="ExternalInput"
    ).ap()
    bT = nc.dram_tensor("bT", [KT, 128, 4, N], bf16, kind="ExternalInput").ap()
    bqk = nc.dram_tensor("bqk", [128, 2 * KT], f32, kind="ExternalInput").ap()
    outT = nc.dram_tensor("outT", [KT, 128, T], bf16, kind="ExternalOutput").ap()


    with tile.TileContext(nc) as tc:
        from contextlib import ExitStack

        with ExitStack() as ctx:
            nc.gpsimd.load_library(library_config.proxy)
            cp = ctx.enter_context(tc.tile_pool(name="consts", bufs=1))
            psA = ctx.enter_context(tc.tile_pool(name="psA", bufs=2, space="PSUM"))
            psC = ctx.enter_context(tc.tile_pool(name="psC", bufs=3, space="PSUM"))
            wp = ctx.enter_context(tc.tile_pool(name="work", bufs=2))

            # ---- persistent SBUF tiles; DMAs in consumption order ----
            xh_sb = cp.tile([128, KT // 2, 2, TQ], f8, name="xh", tag="xh")
            xx_sb = cp.tile([128, KT, 2, TQ], f8, name="xx", tag="xx")
            wvx_sb = cp.tile([128, 2, KT, 2, C // 2], f8, name="wvx", tag="wvx")
            wqkh_sb = cp.tile(
                [128, 2 * KT, KT // 2, 2, 128], f8, name="wqkh", tag="wqkh"
            )
            wqkx_sb = cp.tile(
                [128, 2 * KT, KT, 2, 128], f8, name="wqkx", tag="wqkx"
            )
            bqk_sb = cp.tile([128, 2 * KT], f32, name="bqk", tag="bqk")
            # startup-critical DMAs, spread so each consumer unblocks at
            # its need time: B prologue needs xh0/xx0/wvx; prologue A units
            # need only the mt=0 and mt=6 slices of wqk; later windows and
            # head-pairs stream in during the loop
            nc.gpsimd.dma_start(out=wvx_sb[:, 0], in_=wv_x[0])
            nc.gpsimd.dma_start(out=wvx_sb[:, 1], in_=wv_x[1])
            # batch-0 token span lands first, then batch 1, then the rest
            nc.scalar.dma_start(out=xh_sb[:, :, :, 0:N], in_=x_hi[:, :, :, 0:N])
            nc.sync.dma_start(out=xx_sb[:, :, :, 0:N], in_=x_x[:, :, :, 0:N])
            nc.scalar.dma_start(out=xh_sb[:, :, :, N:TN], in_=x_hi[:, :, :, N:TN])
            nc.sync.dma_start(out=xx_sb[:, :, :, N:TN], in_=x_x[:, :, :, N:TN])
            # wqk mt-slices in hp-need order (both Q and K of pair hp are
            # needed by iteration hp), alternating gpsimd/sync
            for hp in range(KT):
                qa = nc.gpsimd if hp % 2 == 0 else nc.sync
                for m in (hp, KT + hp):
                    qa.dma_start(out=wqkh_sb[:, m], in_=wqk_hi[:, m])
                    qa.dma_start(out=wqkx_sb[:, m], in_=wqk_x[:, m])
            nc.scalar.dma_start(out=bqk_sb[:], in_=bqk[:])
            for nt in range(1, NT):
                o = nt * TN
                e_ = TQ if nt == NT - 1 else (nt + 1) * TN
                nc.sync.dma_start(out=xh_sb[:, :, :, o:e_], in_=x_hi[:, :, :, o:e_])
                nc.sync.dma_start(out=xx_sb[:, :, :, o:e_], in_=x_x[:, :, :, o:e_])
            bias_sb = [
                cp.tile([128, 4, N], bf16, name=f"bias{hp}", tag=f"bias{hp}")
                for hp in range(KT)
            ]
            wpj8_sb = cp.tile(
                [128, 2, KT, KT // 2, 2, 128], f8, name="wpj8", tag="wpj8"
            )
            TP = T + 8  # attn pair-step must be 16-byte aligned
            attnp = [
                cp.tile([128, KT // 2, 2, TP], f8, name=f"atp{kind}", tag=f"atp{kind}")
                for kind in range(2)
            ]

            # qk tiles: Q (mt 0..5), K (mt 6..11)
            qk_sb = [
                cp.tile([128, T], bf16, name=f"qk{m}", tag=f"qk{m}")
                for m in range(2 * KT)
            ]
            # V per (batch, jt): (rows, 12 heads, 65) -- 64 V cols + ones col
            v_sb = {}
            for b in range(BPC):
                for jt, rows in enumerate(JROWS):
                    t_ = cp.tile(
                        [rows, H, D + 1], bf16, name=f"v{b}_{jt}", tag=f"v{b}_{jt}"
                    )
                    nc.vector.memset(t_[:, :, D : D + 1], 1.0)
                    v_sb[(b, jt)] = t_

            evict_flip = [0]

            def evict_engine():
                evict_flip[0] += 1
                return nc.vector if evict_flip[0] % 8 in (0, 3, 6) else nc.scalar

            # ---- filler unit emitters (pure PE work + one eviction) ----
            def unit_b(b, jt, n2, psv=None):
                """V-proj quarter: one psum group -> v_sb[(b, jt)] slice."""
                rows = JROWS[jt]
                o = b * N + jt * 128
                if psv is None:
                    psv = psA.tile([128, 512], f32, tag="psA")
                for p in range(KT // 2):
                    nc.tensor.matmul(
                        psv[0:rows, 0 : C // 2],
                        xh_sb[:, p, :, o : o + rows],
                        wvx_sb[:, n2, 2 * p : 2 * p + 2, 1, :],
                        start=(p == 0),
                        stop=False,
                        perf_mode=DR,
                    )
                for k in range(KT):
                    nc.tensor.matmul(
                        psv[0:rows, 0 : C // 2],
                        xx_sb[:, k, :, o : o + rows],
                        wvx_sb[:, n2, k, :, :],
                        start=False,
                        stop=(k == KT - 1),
                        perf_mode=DR,
                    )
                eng = evict_engine()
                dst = v_sb[(b, jt)][0:rows, n2 * KT : (n2 + 1) * KT, 0:D]
                src = psv[0:rows, 0 : C // 2].rearrange("p (h d) -> p h d", h=KT)
                if eng is nc.vector:
                    nc.vector.tensor_copy(dst, src)
                else:
                    nc.scalar.activation(dst, src, AF.Copy)

            # cross terms for these k-tiles are dropped in the QK proj
            # (error-compensation budget spent for PE time; validated at
            # ~1.5e-2 total rel err vs the 2e-2 gate)
            ADROP = (0, 3)
            AKEEP = tuple(k for k in range(KT) if k not in ADROP)

            def unit_a(mt, nt, eng=None):
                """QK-proj subtile: one token window of Q or K tile mt."""
                inv_s = (1.0 / SQ) if mt < KT else (1.0 / SK)
                o = nt * TN
                ps = psA.tile([128, 512], f32, tag="psA")
                for p in range(KT // 2):
                    nc.tensor.matmul(
                        ps[:, 0:TN],
                        wqkh_sb[:, mt, p, :, :],
                        xh_sb[:, p, :, o : o + TN],
                        start=(p == 0),
                        stop=False,
                        perf_mode=DR,
                    )
                for k in AKEEP:
                    nc.tensor.matmul(
                        ps[:, 0:TN],
                        wqkx_sb[:, mt, k, :, :],
                        xx_sb[:, k, :, o : o + TN],
                        start=False,
                        stop=(k == AKEEP[-1]),
                        perf_mode=DR,
                    )
                dst = qk_sb[mt][:, nt * TN : (nt + 1) * TN]
                srcp = ps[:, 0:TN]
                if (eng or evict_engine()) is nc.vector:
                    nc.vector.tensor_scalar(
                        dst, srcp, inv_s, bqk_sb[:, mt : mt + 1],
                        OP.mult, OP.add,
                    )
                else:
                    nc.scalar.activation(
                        dst, srcp, AF.Identity,
                        bias=bqk_sb[:, mt : mt + 1], scale=inv_s,
                    )

            dq = [0]

            DTERMS = [(0, 0), (1, 0), (0, 1)]  # (weight kind, attn kind)

            def unit_d_mm(ps, nt, mt, ps_):
                """ps_ = list of pair indices p to accumulate (0..2)."""
                for ti, (wk, ak) in enumerate(DTERMS):
                    for p in ps_:
                        nc.tensor.matmul(
                            ps[:, 0:TN],
                            wpj8_sb[:, wk, mt, p, :, :],
                            attnp[ak][:, p, :, nt * TN : (nt + 1) * TN],
                            start=(ti == 0 and p == 0),
                            stop=(ti == 2 and p == 2),
                            perf_mode=DR,
                        )

            def unit_d_fin(ps, nt, mt, act=None):
                ot = wp.tile([128, TN], bf16, tag="ot", bufs=3)
                use_dve = (act is False) if act is not None else (
                    evict_engine() is nc.vector)
                if use_dve:
                    nc.vector.tensor_scalar_mul(ot[:], ps[:, 0:TN], 1.0 / 4096.0)
                else:
                    nc.scalar.activation(ot[:], ps[:, 0:TN], AF.Copy,
                                         scale=1.0 / 4096.0)
                dq[0] += 1
                nc.sync.dma_start(
                    out=outT[mt, :, nt * TN : (nt + 1) * TN], in_=ot[:]
                )

            def unit_d(nt, mt):
                """output-proj group: one (window, out-tile) -> outT DMA."""
                ps = psA.tile([128, 512], f32, tag="psA")
                unit_d_mm(ps, nt, mt, (0, 1, 2))
                unit_d_fin(ps, nt, mt)

            # ---- attention pieces ----
            s_tiles = {}

            def emit_s(g, e2):
                """S^T matmuls + exp + Pool bias-mul, per key-window."""
                b, hp = divmod(g, KT)
                for jt, rows in enumerate(JROWS):
                    ps = psC.tile([128, 2, 512], f32, tag="psC")
                    for hh in range(2):
                        base = 64 * hh
                        i0 = b * N + jt * 128
                        nc.tensor.matmul(
                            ps[0:rows, hh, 0:N],
                            qk_sb[KT + hp][base : base + 64, i0 : i0 + rows],
                            qk_sb[hp][base : base + 64, b * N : (b + 1) * N],
                            start=True,
                            stop=True,
                        )
                    eu = wp.tile([128, 2, N], bf16, tag=f"eu{jt}", bufs=3)
                    nc.scalar.activation(
                        eu[0:rows, :, :], ps[0:rows, :, 0:N], AF.Exp
                    )
                    nc.gpsimd.tensor_mul(
                        e2[0:rows, :, jt, :],
                        eu[0:rows, :, :],
                        bias_sb[hp][0:rows, 2 * jt : 2 * jt + 2, :],
                    )

            def emit_av(g):
                """AV + softmax normalize for iteration g (runs at g+1)."""
                b, hp = divmod(g, KT)
                e2 = s_tiles.pop(g)
                po = psC.tile([128, 2, 512], f32, tag="psC")
                for hh in range(2):
                    h = 2 * hp + hh
                    for jt, rows in enumerate(JROWS):
                        nc.tensor.matmul(
                            po[0 : D + 1, hh, 0:N],
                            v_sb[(b, jt)][0:rows, h, 0 : D + 1],
                            e2[0:rows, hh, jt, :],
                            start=(jt == 0),
                            stop=(jt == 1),
                        )
                # per-head recip/bcast/mul chains pipeline DVE and Pool,
                # shortening the latency until po's PSUM banks free up
                r2 = wp.tile([1, 2, N], bf16, tag="r2", bufs=3)
                rb = wp.tile([128, N2], bf16, tag="rb", bufs=3)
                at = wp.tile([128, N], bf16, tag="atmp", bufs=3)
                with nc.allow_low_precision(
                    reason="softmax denom reciprocal in bf16"
                ):
                    nc.vector.reciprocal(r2[:, 0, :], po[D : D + 1, 0, 0:N])
                    nc.gpsimd.partition_broadcast(rb[:, 0:N], r2[:, 0, :])
                    nc.vector.reciprocal(r2[:, 1, :], po[D : D + 1, 1, 0:N])
                    nc.gpsimd.partition_broadcast(rb[:, N:N2], r2[:, 1, :])
                nc.vector.tensor_mul(at[0:D, :], po[0:D, 0, 0:N], rb[0:D, 0:N])
                nc.vector.tensor_mul(
                    at[D : 2 * D, :], po[0:D, 1, 0:N], rb[D : 2 * D, N:N2]
                )
                # fp8 hi/lo split of attn^T (feeds the DR output proj)
                nc.gpsimd.tensor_copy(
                    attnp[0][:, hp // 2, hp % 2, b * N : (b + 1) * N], at[:]
                )
                nc.gpsimd.tensor_sub(
                    attnp[1][:, hp // 2, hp % 2, b * N : (b + 1) * N],
                    at[:],
                    attnp[0][:, hp // 2, hp % 2, b * N : (b + 1) * N],
                )

            # ---- static filler schedule ----
            # AB units have HARD deadlines (due = first iteration whose S or
            # AV reads their output; emitting later would cycle the in-order
            # PE queue through an ACT/DVE eviction that sits behind stalled
            # work -> deadlock).  D units instead have a READY iteration
            # (earliest emission keeping attn writes ahead in PE order).
            abunits = []
            for b in range(2, BPC):
                for jt in range(2):
                    for n2 in range(2):
                        # V[b] consumed by AV[b, hp0] emitted at iter 6b+1
                        abunits.append(
                            (max(0, KT * b - 1),
                             lambda b=b, jt=jt, n2=n2: unit_b(b, jt, n2))
                        )
            for hp in range(KT):
                for nt in range(NT):
                    for mt in (hp, KT + hp):
                        if hp == 0 and nt == 0:
                            continue  # prologue
                        # first S reading window nt of pair hp: b = 2nt
                        abunits.append(
                            (2 * nt * KT + hp,
                             lambda mt=mt, nt=nt: unit_a(mt, nt))
                        )
            abunits.sort(key=lambda u: u[0])
            # D(nt) is ready from iteration 12nt+13 (window nt's last norms
            # are emitted during iteration 12nt+12).  Schedule the units as
            # LATE as allowed: the endgame iterations (>=42) have no A/B
            # units left (all deadlines passed) and otherwise starve PE.
            DSLOT = {0: (15, 17, 19, 21, 23, 25),
                     1: (27, 30, 33, 36, 39, 42),
                     2: (43, 44, 45, 46, 47, 47)}
            DSLOT = {0: (15, 17, 19, 21, 23, 25),
                     1: (27, 30, 34, 38, 42, 44),
                     2: (44, 45, 46, 46, 47, 47)}
            dsched = {}
            for nt in range(NT - 1):
                for mt in range(KT):
                    dsched.setdefault(DSLOT[nt][mt], []).append(
                        lambda nt=nt, mt=mt: unit_d(nt, mt)
                    )

            # ---- prologue ----
            for hp in range(KT):
                (nc.scalar if hp % 2 == 0 else nc.gpsimd).dma_start(
                    out=bias_sb[hp][:], in_=bT[hp]
                )
            nc.sync.dma_start(out=wpj8_sb[:, 0], in_=wpj8[0])
            nc.sync.dma_start(out=wpj8_sb[:, 1], in_=wpj8[1])
            # prologue B units run 2-per-psC-tile (6 banks idle here), so
            # PE isn't paced by the psA double-buffer eviction latency
            for b in range(2):
                for jt in range(2):
                    t_ = psC.tile([128, 2, 512], f32, tag="psC")
                    for n2 in range(2):
                        unit_b(b, jt, n2, psv=t_[:, n2])

            unit_a(0, 0)
            unit_a(KT, 0)

            # ---- flat pipelined loop ----
            NITER = KT * BPC
            TARGET = 1400  # ns of PE filler per iteration
            ABCOST = 700   # avg A/B unit PE ns
            ai = 0
            for g in range(NITER + 1):
                # hard-due AB units must precede S[g]
                spent = 0
                while ai < len(abunits) and abunits[ai][0] <= g:
                    abunits[ai][1]()
                    ai += 1
                    spent += ABCOST
                if g < NITER:
                    e2 = wp.tile([128, 2, 2, N], bf16, tag="e2", bufs=3)
                    s_tiles[g] = e2
                    emit_s(g, e2)
                # scheduled D units + EDF top-up between S[g] and AV[g-1]
                for fn in dsched.get(g, ()):
                    fn()
                    spent += 738
                while spent < TARGET and ai < len(abunits):
                    abunits[ai][1]()
                    ai += 1
                    spent += ABCOST
                if g == NITER:
                    # last window: pair p=0,1 partials only need head-pairs
                    # 0..3; they fill PE while the final AV/normalize chain
                    # completes.  Two extra 2-bank psC tiles host one output
                    # tile's group per bank.
                    d3 = []
                    for mt in range(2):
                        t_ = psA.tile([128, 512], f32, tag="psA")
                        d3.append(t_[:])
                        unit_d_mm(t_[:], NT - 1, mt, (0, 1))
                if g >= 1:
                    emit_av(g - 1)
            # post-AV drain: two more p01 partials (psC banks are free once
            # the last exp retires) fill PE while the final normalize/fp8
            # split chain completes
            d3c = psC.tile([128, 2, 512], f32, tag="psC")
            for bk in range(2):
                unit_d_mm(d3c[:, bk], NT - 1, 2 + bk, (0, 1))
            for mt in range(2):
                unit_d_mm(d3[mt], NT - 1, mt, (2,))
                unit_d_fin(d3[mt], NT - 1, mt, act=True)
            for bk in range(2):
                unit_d_mm(d3c[:, bk], NT - 1, 2 + bk, (2,))
                unit_d_fin(d3c[:, bk], NT - 1, 2 + bk, act=True)
            for mt in range(4, KT):
                ps = psA.tile([128, 512], f32, tag="psA")
                unit_d_mm(ps, NT - 1, mt, (0, 1, 2))
                unit_d_fin(ps, NT - 1, mt, act=True)

    if finalize:
        nc.finalize()
    return nc


def _split8(a):
    """Error-compensated fp8 pair: a ~= hi + lo, each e4m3."""
    hi = a.astype(F8)
    lo = (a - hi.astype(np.float32)).astype(F8)
    return hi, lo


def _ktiles(a, nf):
    """(768, nf) -> (128, KT, nf) partition-major k-tiles."""
    return np.ascontiguousarray(a.reshape(KT, 128, nf).transpose(1, 0, 2))


def _host_prep(x, qkv_w, qkv_b, proj_w, proj_b, rel_table, log_temp, rel_index):
    """Build the per-core input maps (host-side layout prep only)."""
    x = np.asarray(x, np.float32)
    qkv_w = np.asarray(qkv_w, np.float32)
    qkv_b = np.asarray(qkv_b, np.float32)
    proj_w = np.asarray(proj_w, np.float32)
    rel_table = np.asarray(rel_table, np.float32)
    log_temp = np.asarray(log_temp, np.float32)
    rel_index = np.asarray(rel_index)

    temp = np.log1p(np.exp(log_temp.astype(np.float64))).astype(np.float32)
    alpha = (SCALE / temp).astype(np.float32)         # (H,) folded into q
    alpha_c = np.repeat(alpha, D)                     # (768,)

    # qk weights, host-scaled for fp8 range (SQ incl. alpha; SK plain),
    # split into hi/lo e4m3 pairs; hi-only and interleaved-cross layouts
    wqkT = qkv_w[0 : 2 * C].T.copy()                  # (768, 1536)
    wqkT[:, 0:C] *= alpha_c[None, :] * SQ
    wqkT[:, C : 2 * C] *= SK
    qhi, qlo = _split8(wqkT)
    qhi_t = _ktiles(qhi.astype(np.float32), 2 * C)
    qlo_t = _ktiles(qlo.astype(np.float32), 2 * C)
    wqk_hi_np = np.ascontiguousarray(
        qhi_t.reshape(128, KT // 2, 2, 2 * KT, 128).transpose(0, 3, 1, 2, 4)
    ).astype(F8)
    # cross weights LO-FIRST so the shared x cross buffer can stay HI-FIRST
    wqk_x_np = np.ascontiguousarray(
        np.stack([qlo_t, qhi_t], axis=2)
        .reshape(128, KT, 2, 2 * KT, 128)
        .transpose(0, 3, 1, 2, 4)
    ).astype(F8)

    # wv as fp8 hi/lo cross pairs, LO-FIRST (moving operand of V phase),
    # host-scaled by SVF out of the e4m3 subnormal range
    wvT = qkv_w[2 * C : 3 * C].T * SVF                # (768, 768)
    vhi, vlo = _split8(wvT)
    vhi_t = _ktiles(vhi.astype(np.float32), C)
    vlo_t = _ktiles(vlo.astype(np.float32), C)
    wvx = np.stack([vlo_t, vhi_t], axis=2)            # (128, KT, 2, C) lo-first
    wv_x_np = np.stack(
        [wvx[:, :, :, 0 : C // 2], wvx[:, :, :, C // 2 : C]], axis=0
    ).astype(F8)
    wpjT = proj_w.T * SP                              # (768, 768)
    phi, plo = _split8(wpjT)
    phi_t = _ktiles(phi.astype(np.float32), C).reshape(128, KT // 2, 2, C)
    plo_t = _ktiles(plo.astype(np.float32), C).reshape(128, KT // 2, 2, C)
    wpj8_np = np.stack(
        [
            np.ascontiguousarray(
                t.reshape(128, KT // 2, 2, KT, 128).transpose(0, 3, 1, 2, 4)
            )
            for t in (phi_t, plo_t)
        ],
        axis=0,
    ).astype(F8)

    bq = qkv_b[0:C] * alpha_c
    bk = qkv_b[C : 2 * C]
    bqk_np = np.concatenate([bq, bk]).reshape(2 * KT, 128).T.copy().astype(np.float32)

    # multiplicative bias table: exp((relpos bias)/temp), diag -> 0, CLS -> 1,
    # transposed to (j, i); paired layout (KT, j, 2N)
    rpb = rel_table[rel_index]                        # (196, 196, H)
    bias = np.zeros((H, N, N), np.float32)
    bias[:, 1:, 1:] = rpb.transpose(2, 0, 1) / temp[:, None, None]
    ebias = np.exp(bias)
    idx = np.arange(1, N)
    ebias[:, idx, idx] = 0.0
    ebT = ebias.transpose(0, 2, 1)                    # (H, j, i)
    bT_np = np.zeros((KT, 128, 4, N), np.float32)
    for jt, rows in enumerate(JROWS):
        blk = ebT[:, jt * 128 : jt * 128 + rows, :]   # (H, rows, N)
        bT_np[:, 0:rows, 2 * jt : 2 * jt + 2] = (
            blk.reshape(KT, 2, rows, N).transpose(0, 2, 1, 3)
        )
    bT_np = bT_np.astype(BF16)


    in_maps = []
    for c in range(NCORES):
        xc = x[c * BPC : (c + 1) * BPC].reshape(T, C).T  # (768, T)
        xhi, xlo = _split8(xc)
        xhi_t = _ktiles(xhi.astype(np.float32), T)      # (128, KT, T)
        xlo_t = _ktiles(xlo.astype(np.float32), T)
        x_hi_np = np.zeros((128, KT // 2, 2, TQ), np.float32)
        x_hi_np[:, :, :, 0:T] = xhi_t.reshape(128, KT // 2, 2, T)
        x_hi_np = x_hi_np.astype(F8)
        x_x_np = np.zeros((128, KT, 2, TQ), np.float32)
        x_x_np[:, :, :, 0:T] = np.stack([xhi_t, xlo_t], axis=2)
        x_x_np = x_x_np.astype(F8)
        in_maps.append(
            {
                "x_hi": x_hi_np,
                "x_x": x_x_np,
                "wv_x": wv_x_np,
                "wqk_hi": wqk_hi_np,
                "wqk_x": wqk_x_np,
                "wpj8": wpj8_np,
                "bT": bT_np,
                "bqk": bqk_np,
            }
        )
    return in_maps


def kernel(**inputs) -> np.ndarray:
    global LAST_RESULTS
    from concourse.bass_utils import run_bass_kernel_spmd

    if "nc" not in _CACHE:
        _CACHE["nc"] = _build()
    nc = _CACHE["nc"]

    in_maps = _host_prep(**inputs)
    try:
        res = run_bass_kernel_spmd(
            nc, in_maps, core_ids=list(range(NCORES)), trace=TRACE
        )
    except ModuleNotFoundError:
        res = run_bass_kernel_spmd(
            nc, in_maps, core_ids=list(range(NCORES)), trace=False
        )
    LAST_RESULTS = res

    # v-bias rides through attention unchanged (rows of attn sum to 1), so
    # its proj image folds into the constant output bias added here
    proj_b = np.asarray(inputs["proj_b"], np.float32)
    proj_w = np.asarray(inputs["proj_w"], np.float32)
    bv = np.asarray(inputs["qkv_b"], np.float32)[2 * C : 3 * C]
    b_eff = proj_b + proj_w @ bv
    outs = []
    for c in range(NCORES):
        oT = np.asarray(res.results[c]["outT"], np.float32).reshape(C, T)
        outs.append(oT.T.reshape(BPC, N, C))
    out = np.concatenate(outs, axis=0) + b_eff[None, None, :]
    return out.astype(np.float32)
